# revision 5
# baseline (speedup 1.0000x reference)
"""GCN layer (out = D^-1/2 (A+I) D^-1/2 (x W^T + b)) on 8 trn2 NeuronCores.

Strategy (v2 — direct-x aggregation, no intermediate table):
  out[dst] = W @ (sum_e norm_e * x[src_e]) + (sum_e norm_e) * b
  with norm_e = rdeg[src]*rdeg[dst].

  - Host: append self-loops, sort edges by dst, partition dst-blocks (128
    nodes) across 8 cores (LPT), build a core-invariant static schedule:
    per (slot, subtable) segments padded at LANE level to the max across
    cores (uniform SPMD program; per-core idx/weight tables).
  - Device: gather raw x rows (bf16, 256B) per edge lane straight from the
    uploaded input table (4 int16-indexable subtable views); accumulate
    aggT[din, dst] = sum_lane x_lane[din] * S[lane, dst] on the PE, where
    S = weighted one-hot (weight = norm_e, built by DVE is_equal * w);
    then one W matmul per dst block; bias term applied on host
    (out += rdeg*sraw*b, with sraw = sum_e rdeg[src]).

  vs v1: removes the whole P1 phase (x stream + T-table write, ~215us of
  modeled DMA) and ~18% gather lane fragmentation.
"""

import math
import time
from contextlib import ExitStack

import ml_dtypes
import numpy as np

import concourse.bass as bass
import concourse.tile as tile
from concourse import bacc, mybir
from concourse.bass_utils import run_bass_kernel_spmd

F32 = mybir.dt.float32
BF16 = mybir.dt.bfloat16
I16 = mybir.dt.int16

N_NODES = 100000
N_EDGES = 1600000
IN_CH = 128
OUT_CH = 128
N_CORES = 8

# ---------------------------------------------------------------------------
# Host-side planning
# ---------------------------------------------------------------------------


class Plan:
    pass


def build_plan(src_all, dst_all, n_nodes, n_cores, d=128, gb=32, sg=8, ob=8,
               subt_cap=32768):
    """src_all/dst_all: edge endpoints INCLUDING self loops."""
    t0 = time.time()
    p = Plan()
    p.d = d
    p.gb = gb
    p.sg = sg
    p.ob = ob
    p.n_nodes = n_nodes
    p.n_cores = n_cores

    n_blocks = math.ceil(n_nodes / 128)
    slots = math.ceil(n_blocks / n_cores)
    n_blocks = slots * n_cores
    p.slots = slots
    p.n_blocks = n_blocks
    p.n_pad = n_blocks * 128

    # subtables over node id range (int16 gather index limit)
    n_subt = math.ceil(p.n_pad / subt_cap)
    off = [min(q * subt_cap, p.n_pad) for q in range(n_subt + 1)]
    p.subt_off = np.asarray(off)
    p.n_subt = n_subt

    src_all = np.asarray(src_all, dtype=np.int64)
    dst_all = np.asarray(dst_all, dtype=np.int64)

    # degrees / norms (host; exactly mirrors the reference formula)
    deg = np.bincount(dst_all, minlength=p.n_pad).astype(np.float64)
    deg[n_nodes:] = 1.0
    deg[deg == 0] = 1.0
    rdeg = 1.0 / np.sqrt(deg)
    p.rdeg = rdeg.astype(np.float32)
    norm_all = (rdeg[src_all] * rdeg[dst_all]).astype(np.float32)
    # host bias epilogue: out[v] += rdeg[v] * sraw[v] * b
    sraw = np.zeros(p.n_pad, dtype=np.float64)
    np.add.at(sraw, dst_all, rdeg[src_all])
    p.sraw = sraw.astype(np.float32)

    # sort edges by dst
    order = np.argsort(dst_all, kind="stable")
    dst_s = dst_all[order]
    src_s = src_all[order]
    norm_s = norm_all[order]
    blk_ptr = np.searchsorted(dst_s, np.arange(0, p.n_pad + 1, 128))
    blk_cnt = blk_ptr[1:] - blk_ptr[:-1]

    # per (block, q) edge sublists
    subt_of_src = np.searchsorted(p.subt_off[1:], src_s, side="right")
    blk_edges = []  # [block][q] -> (src_local, dst_local, norm)
    blk_q_cnt = np.zeros((n_blocks, n_subt), dtype=np.int64)
    for b in range(n_blocks):
        lo, hi = blk_ptr[b], blk_ptr[b + 1]
        qs = subt_of_src[lo:hi]
        per_q = []
        for q in range(n_subt):
            m = qs == q
            sl = (src_s[lo:hi][m] - p.subt_off[q]).astype(np.int16)
            dl = (dst_s[lo:hi][m] - b * 128).astype(np.int32)
            nm = norm_s[lo:hi][m]
            per_q.append((sl, dl, nm))
            blk_q_cnt[b, q] = len(sl)
        blk_edges.append(per_q)

    # Slot grouping: blocks sharing a slot should have near-identical per-q
    # edge counts, since seg_len[s][q] = max over the group. Self-loops
    # concentrate ~128 edges in a block's home subtable, so group by
    # (home subtable, size rank); within a slot, biggest block goes to the
    # least-loaded core (balances per-core totals).
    home = np.searchsorted(p.subt_off[1:], np.arange(n_blocks) * 128,
                           side="right")
    groups = []
    for q in range(n_subt):
        hb = np.where(home == q)[0]
        hb = hb[np.argsort(-blk_cnt[hb], kind="stable")]
        assert len(hb) % n_cores == 0, (q, len(hb))
        for i in range(0, len(hb), n_cores):
            groups.append(hb[i:i + n_cores])
    assert len(groups) == slots
    core_loads = np.zeros(n_cores, dtype=np.int64)
    core_blocks = [[-1] * slots for _ in range(n_cores)]
    for s, group in enumerate(groups):
        free = list(range(n_cores))
        for b in group:  # already desc by size
            c = min(free, key=lambda c: core_loads[c])
            free.remove(c)
            core_loads[c] += int(blk_cnt[b])
            core_blocks[c][s] = int(b)
    p.core_blocks = core_blocks

    # lane-level segments: seg_len[s][q] = max over cores
    seg_len = np.zeros((slots, n_subt), dtype=np.int64)
    for c in range(n_cores):
        for s in range(slots):
            seg_len[s] = np.maximum(seg_len[s], blk_q_cnt[core_blocks[c][s]])
    p.seg_len = seg_len
    seg_start = np.zeros((slots, n_subt), dtype=np.int64)
    cur = np.zeros(n_subt, dtype=np.int64)
    for s in range(slots):
        seg_start[s] = cur
        cur += seg_len[s]
    p.seg_start = seg_start
    stream_len = [int(-(-int(cur[q]) // 128) * 128) for q in range(n_subt)]
    p.stream_len = stream_len
    p.n_batches = [math.ceil(sl / (gb * 128)) if sl else 0 for sl in stream_len]

    # instance enumeration (program order: slot-major, q, column)
    insts = []
    slot_inst_range = [0]
    inst_index = {}
    for s in range(slots):
        cnt = 0
        for q in range(n_subt):
            L0 = int(seg_start[s][q])
            L1 = L0 + int(seg_len[s][q])
            if L1 > L0:
                for col in range(L0 // 128, -(-L1 // 128)):
                    inst_index[(s, q, col)] = len(insts)
                    insts.append((s, q, col))
                    cnt += 1
        if cnt == 0:  # fully-empty slot across all cores (pad blocks)
            col = min(int(seg_start[s][0]) // 128,
                      max(stream_len[0] // 128 - 1, 0))
            inst_index[(s, 0, col)] = len(insts)
            insts.append((s, 0, col))
            cnt = 1
        slot_inst_range.append(len(insts))
    p.n_inst = len(insts)
    p.inst_slot = np.asarray([i[0] for i in insts], dtype=np.int64)
    p.inst_q = np.asarray([i[1] for i in insts], dtype=np.int64)
    p.inst_col = np.asarray([i[2] for i in insts], dtype=np.int64)
    p.slot_inst_range = slot_inst_range

    # per-core tables
    p.core_idx = []      # [n_cores][q] int16 wrapped [128, stream_len/16]
    p.core_dst_rel = []  # [n_cores] bf16 [128, n_inst]
    p.core_w = []        # [n_cores] bf16 [128, n_inst]
    for c in range(n_cores):
        idx_q = [np.zeros(stream_len[q], dtype=np.int16)
                 for q in range(n_subt)]
        dst_rel = np.full((128, p.n_inst), -1.0, dtype=np.float32)
        w_lane = np.zeros((128, p.n_inst), dtype=np.float32)
        for s in range(slots):
            b = core_blocks[c][s]
            for q in range(n_subt):
                sl, dl, nm = blk_edges[b][q]
                n_e = len(sl)
                if n_e == 0:
                    continue
                L0 = int(seg_start[s][q])
                idx_q[q][L0:L0 + n_e] = sl
                lanes = L0 + np.arange(n_e)
                cols = lanes // 128
                rows = lanes % 128
                iis = np.asarray([inst_index[(s, q, int(cc))] for cc in cols],
                                 dtype=np.int64)
                dst_rel[rows, iis] = dl
                w_lane[rows, iis] = nm
        idx_wrapped = []
        for q in range(n_subt):
            if stream_len[q] == 0:
                idx_wrapped.append(np.zeros((128, 1), dtype=np.int16))
                continue
            a = idx_q[q].reshape(-1, 16).T  # [16, L/16]
            idx_wrapped.append(np.tile(a, (8, 1)).copy())
        p.core_idx.append(idx_wrapped)
        p.core_dst_rel.append(dst_rel.astype(ml_dtypes.bfloat16))
        p.core_w.append(w_lane.astype(ml_dtypes.bfloat16))

    p.plan_time = time.time() - t0
    return p


# ---------------------------------------------------------------------------
# Device kernel
# ---------------------------------------------------------------------------


def build_nc(p, n_cores=None):
    d = p.d
    gb, sg, ob = p.gb, p.sg, p.ob

    nc = bacc.Bacc("TRN2", target_bir_lowering=False, debug=False,
                   num_devices=n_cores or p.n_cores)

    X = nc.dram_tensor("X", [p.n_pad, d], BF16, kind="ExternalInput")
    WT = nc.dram_tensor("WT", [d, d], BF16, kind="ExternalInput")
    iota = nc.dram_tensor("iota", [128, 128], BF16, kind="ExternalInput")
    dst_rel = nc.dram_tensor("dst_rel", [128, p.n_inst], BF16,
                             kind="ExternalInput")
    w_t = nc.dram_tensor("w_t", [128, p.n_inst], BF16, kind="ExternalInput")
    idx_t = [nc.dram_tensor(f"idx{q}", [128, max(p.stream_len[q] // 16, 1)],
                            I16, kind="ExternalInput")
             for q in range(p.n_subt)]
    out_t = nc.dram_tensor("out", [d, p.slots * 128], F32,
                           kind="ExternalOutput")

    with tile.TileContext(nc) as tc, ExitStack() as ctx:
        cpool = ctx.enter_context(tc.tile_pool(name="consts", bufs=1))
        WT_sb = cpool.tile([d, d], BF16)
        nc.sync.dma_start(WT_sb[:], WT.ap()[:, :])
        iota_sb = cpool.tile([128, 128], BF16)
        nc.sync.dma_start(iota_sb[:], iota.ap()[:, :])
        dstrel_sb = cpool.tile([128, p.n_inst], BF16)
        nc.sync.dma_start(dstrel_sb[:], dst_rel.ap()[:, :])
        w_sb = cpool.tile([128, p.n_inst], BF16)
        nc.sync.dma_start(w_sb[:], w_t.ap()[:, :])
        idx_sb = []
        for q in range(p.n_subt):
            t = cpool.tile([128, idx_t[q].shape[1]], I16, name=f"idxsb{q}")
            nc.sync.dma_start(t[:], idx_t[q].ap()[:, :])
            idx_sb.append(t)

        gpools = [ctx.enter_context(tc.tile_pool(name=f"g{q}", bufs=2))
                  for q in range(p.n_subt)]
        rawpool = ctx.enter_context(tc.tile_pool(name="raw", bufs=2))
        stpool = ctx.enter_context(tc.tile_pool(name="st", bufs=3))
        aggpool = ctx.enter_context(tc.tile_pool(name="agg", bufs=3))
        opool = ctx.enter_context(tc.tile_pool(name="ostage", bufs=2))
        pa = ctx.enter_context(tc.tile_pool(name="pa", bufs=4, space="PSUM"))
        pb = ctx.enter_context(tc.tile_pool(name="pb", bufs=4, space="PSUM"))

        gtiles = {}

        def get_gtile(q, i):
            if (q, i) not in gtiles:
                lanes = min(gb * 128, p.stream_len[q] - i * gb * 128)
                gt = gpools[q].tile([128, gb * d], BF16, name=f"gt{q}")
                nc.gpsimd.dma_gather(
                    out_ap=gt[:, 0:(lanes // 128) * d].rearrange(
                        "p (j d) -> p j d", d=d),
                    in_ap=X.ap()[int(p.subt_off[q]):int(p.subt_off[q + 1]), :],
                    idxs_ap=idx_sb[q][:, i * gb * 8: i * gb * 8 + lanes // 16],
                    num_idxs=lanes,
                    num_idxs_reg=lanes,
                    elem_size=d,
                    single_packet=False,
                )
                gtiles[(q, i)] = gt
            return gtiles[(q, i)]

        st_cur = None
        ostage = None
        for s in range(p.slots):
            ps_a = pa.tile([128, d], F32, name="pa_t", space="PSUM")
            i0, i1 = p.slot_inst_range[s], p.slot_inst_range[s + 1]
            for ii in range(i0, i1):
                if ii % sg == 0:
                    ng = min(sg, p.n_inst - ii)
                    raw = rawpool.tile([128, sg * 128], BF16, name="raw_t")
                    st_cur = stpool.tile([128, sg * 128], BF16, name="st_t")
                    nc.vector.tensor_tensor(
                        out=raw[:, 0:ng * 128].rearrange(
                            "p (g i) -> p g i", i=128),
                        in0=dstrel_sb[:, ii:ii + ng].unsqueeze(2).broadcast_to(
                            (128, ng, 128)),
                        in1=iota_sb[:].unsqueeze(1).broadcast_to(
                            (128, ng, 128)),
                        op=mybir.AluOpType.is_equal)
                    nc.vector.tensor_tensor(
                        out=st_cur[:, 0:ng * 128].rearrange(
                            "p (g i) -> p g i", i=128),
                        in0=raw[:, 0:ng * 128].rearrange(
                            "p (g i) -> p g i", i=128),
                        in1=w_sb[:, ii:ii + ng].unsqueeze(2).broadcast_to(
                            (128, ng, 128)),
                        op=mybir.AluOpType.mult)
                q = int(p.inst_q[ii])
                c = int(p.inst_col[ii])
                gt = get_gtile(q, c // gb)
                o = (c % gb) * d
                so = (ii % sg) * 128
                nc.tensor.matmul(out=ps_a[:], lhsT=gt[:, o:o + d],
                                 rhs=st_cur[:, so:so + 128],
                                 start=(ii == i0), stop=(ii == i1 - 1))
            agg = aggpool.tile([128, d], BF16, name="agg_t")
            if s % 2 == 0:
                nc.scalar.activation(agg[:], ps_a[:],
                                     mybir.ActivationFunctionType.Copy)
            else:
                nc.vector.tensor_scalar_mul(agg[:], ps_a[:], 1.0)
            ps_b = pb.tile([128, d], F32, name="pb_t", space="PSUM")
            nc.tensor.matmul(out=ps_b[:], lhsT=WT_sb[:], rhs=agg[:],
                             start=True, stop=True)
            if s % ob == 0:
                ostage = opool.tile([128, ob * d], F32, name="ostage")
            ocol = (s % ob) * d
            if s % 2 == 0:
                nc.vector.tensor_scalar_mul(ostage[:, ocol:ocol + d], ps_b[:],
                                            1.0)
            else:
                nc.scalar.activation(ostage[:, ocol:ocol + d], ps_b[:],
                                     mybir.ActivationFunctionType.Copy)
            if s % ob == ob - 1 or s == p.slots - 1:
                s0 = s // ob * ob
                nsw = s - s0 + 1
                nc.sync.dma_start(
                    out=out_t.ap()[:, s0 * 128:(s0 + nsw) * 128],
                    in_=ostage[:, 0:nsw * d])

    nc.compile()
    return nc


# ---------------------------------------------------------------------------
# Orchestration
# ---------------------------------------------------------------------------


def make_inputs(p, x, W):
    d = p.d
    Xb = np.zeros((p.n_pad, d), dtype=ml_dtypes.bfloat16)
    Xb[:p.n_nodes] = np.asarray(x, dtype=np.float32).astype(ml_dtypes.bfloat16)
    WT = np.ascontiguousarray(np.asarray(W, dtype=np.float32).T).astype(
        ml_dtypes.bfloat16)
    iota = np.broadcast_to(np.arange(128, dtype=np.float32),
                           (128, 128)).astype(ml_dtypes.bfloat16).copy()
    common = {"X": Xb, "WT": WT, "iota": iota}
    in_maps = []
    for c in range(p.n_cores):
        m = dict(common)
        m["dst_rel"] = p.core_dst_rel[c]
        m["w_t"] = p.core_w[c]
        for q in range(p.n_subt):
            m[f"idx{q}"] = p.core_idx[c][q]
        in_maps.append(m)
    return in_maps


def assemble_output(p, results, b):
    out = np.zeros((p.n_nodes, p.d), dtype=np.float32)
    for c in range(p.n_cores):
        oc = np.asarray(results[c]["out"], dtype=np.float32)  # [d, slots*128]
        for s, blk in enumerate(p.core_blocks[c]):
            lo = blk * 128
            if lo >= p.n_nodes:
                continue
            hi = min(lo + 128, p.n_nodes)
            out[lo:hi] = oc[:, s * 128: s * 128 + (hi - lo)].T
    # bias epilogue: out[v] += rdeg[v] * sraw[v] * b
    coef = (p.rdeg[:p.n_nodes] * p.sraw[:p.n_nodes]).astype(np.float32)
    out += coef[:, None] * np.asarray(b, dtype=np.float32)[None, :]
    return out


def gcn_forward(x, edge_index, W, b, n_cores=8, trace=False, **plan_kw):
    n = x.shape[0]
    src = np.asarray(edge_index[0])
    dst = np.asarray(edge_index[1])
    loop = np.arange(n, dtype=src.dtype)
    src_all = np.concatenate([src, loop])
    dst_all = np.concatenate([dst, loop])
    p = build_plan(src_all, dst_all, n, n_cores, d=W.shape[0], **plan_kw)
    nc = build_nc(p)
    in_maps = make_inputs(p, x, W)
    res = run_bass_kernel_spmd(nc, in_maps, core_ids=list(range(n_cores)),
                               trace=trace)
    out = assemble_output(p, [r for r in res.results], b)
    return out, p, res


def kernel(x, edge_index, W, b):
    """GCN layer forward on 8 trn2 NeuronCores. Inputs as in setup_inputs()."""
    x = np.asarray(x, dtype=np.float32)
    edge_index = np.asarray(edge_index)
    W = np.asarray(W, dtype=np.float32)
    b = np.asarray(b, dtype=np.float32)
    out, _p, _res = gcn_forward(x, edge_index, W, b, n_cores=N_CORES)
    return out.astype(np.float32)


# revision 11
# speedup vs baseline: 1.8002x; 1.8002x over previous
"""GCN layer (out = D^-1/2 (A+I) D^-1/2 (x W^T + b)) on 8 trn2 NeuronCores.

Strategy (v2 — direct-x aggregation, no intermediate table):
  out[dst] = W @ (sum_e norm_e * x[src_e]) + (sum_e norm_e) * b
  with norm_e = rdeg[src]*rdeg[dst].

  - Host: append self-loops, sort edges by dst, partition dst-blocks (128
    nodes) across 8 cores (LPT), build a core-invariant static schedule:
    per (slot, subtable) segments padded at LANE level to the max across
    cores (uniform SPMD program; per-core idx/weight tables).
  - Device: gather raw x rows (bf16, 256B) per edge lane straight from the
    uploaded input table (4 int16-indexable subtable views); accumulate
    aggT[din, dst] = sum_lane x_lane[din] * S[lane, dst] on the PE, where
    S = weighted one-hot (weight = norm_e, built by DVE is_equal * w);
    then one W matmul per dst block; bias term applied on host
    (out += rdeg*sraw*b, with sraw = sum_e rdeg[src]).

  vs v1: removes the whole P1 phase (x stream + T-table write, ~215us of
  modeled DMA) and ~18% gather lane fragmentation.
"""

import math
import time
from contextlib import ExitStack

import ml_dtypes
import numpy as np

import concourse.bass as bass
import concourse.tile as tile
from concourse import bacc, mybir
from concourse.bass_utils import run_bass_kernel_spmd

F32 = mybir.dt.float32
BF16 = mybir.dt.bfloat16
I16 = mybir.dt.int16

N_NODES = 100000
N_EDGES = 1600000
IN_CH = 128
OUT_CH = 128
N_CORES = 8

# ---------------------------------------------------------------------------
# Host-side planning
# ---------------------------------------------------------------------------


class Plan:
    pass


def _optimize_groups(blk_q_cnt, groups, home, n_cores, iters=300000, seed=7):
    """Local search: swap blocks between slot-groups of the same home class
    to reduce sum over groups of sum_q max (the padded lane count)."""
    rng = np.random.default_rng(seed)
    arr = [np.asarray(g).copy() for g in groups]
    G = len(arr)
    ghome = np.asarray([home[g[0]] for g in arr])
    obj = np.asarray([blk_q_cnt[g].max(axis=0).sum() for g in arr])
    # candidate group pairs must share a home class
    byhome = {}
    for i in range(G):
        byhome.setdefault(int(ghome[i]), []).append(i)
    classes = [np.asarray(v) for v in byhome.values() if len(v) > 1]
    if not classes:
        return arr
    weights = np.asarray([len(c) for c in classes], dtype=np.float64)
    weights /= weights.sum()
    ridx = rng.integers(0, n_cores, size=(iters, 2))
    rcls = rng.choice(len(classes), size=iters, p=weights)
    for t in range(iters):
        cls = classes[rcls[t]]
        i, j = cls[rng.integers(0, len(cls), size=2)]
        if i == j:
            continue
        a, b = ridx[t]
        gi, gj = arr[i].copy(), arr[j].copy()
        gi[a], gj[b] = arr[j][b], arr[i][a]
        oi = blk_q_cnt[gi].max(axis=0).sum()
        oj = blk_q_cnt[gj].max(axis=0).sum()
        if oi + oj < obj[i] + obj[j]:
            arr[i], arr[j] = gi, gj
            obj[i], obj[j] = oi, oj
    return arr


def build_plan(src_all, dst_all, n_nodes, n_cores, d=128, gb=32, sg=8, ob=8,
               subt_cap=32768):
    """src_all/dst_all: edge endpoints INCLUDING self loops."""
    t0 = time.time()
    p = Plan()
    p.d = d
    p.gb = gb
    p.sg = sg
    p.ob = ob
    p.n_nodes = n_nodes
    p.n_cores = n_cores

    n_blocks = math.ceil(n_nodes / 128)
    slots = math.ceil(n_blocks / n_cores)
    n_blocks = slots * n_cores
    p.slots = slots
    p.n_blocks = n_blocks
    p.n_pad = n_blocks * 128

    # subtables over node id range (int16 gather index limit)
    n_subt = math.ceil(p.n_pad / subt_cap)
    off = [min(q * subt_cap, p.n_pad) for q in range(n_subt + 1)]
    p.subt_off = np.asarray(off)
    p.n_subt = n_subt

    src_all = np.asarray(src_all, dtype=np.int64)
    dst_all = np.asarray(dst_all, dtype=np.int64)

    # degrees / norms (host; exactly mirrors the reference formula)
    deg = np.bincount(dst_all, minlength=p.n_pad).astype(np.float64)
    deg[n_nodes:] = 1.0
    deg[deg == 0] = 1.0
    rdeg = 1.0 / np.sqrt(deg)
    p.rdeg = rdeg.astype(np.float32)
    norm_all = (rdeg[src_all] * rdeg[dst_all]).astype(np.float32)
    # host bias epilogue: out[v] += rdeg[v] * sraw[v] * b
    sraw = np.zeros(p.n_pad, dtype=np.float64)
    np.add.at(sraw, dst_all, rdeg[src_all])
    p.sraw = sraw.astype(np.float32)

    # sort edges by dst
    order = np.argsort(dst_all, kind="stable")
    dst_s = dst_all[order]
    src_s = src_all[order]
    norm_s = norm_all[order]
    blk_ptr = np.searchsorted(dst_s, np.arange(0, p.n_pad + 1, 128))
    blk_cnt = blk_ptr[1:] - blk_ptr[:-1]

    # per (block, q) edge sublists
    subt_of_src = np.searchsorted(p.subt_off[1:], src_s, side="right")
    blk_edges = []  # [block][q] -> (src_local, dst_local, norm)
    blk_q_cnt = np.zeros((n_blocks, n_subt), dtype=np.int64)
    for b in range(n_blocks):
        lo, hi = blk_ptr[b], blk_ptr[b + 1]
        qs = subt_of_src[lo:hi]
        per_q = []
        for q in range(n_subt):
            m = qs == q
            sl = (src_s[lo:hi][m] - p.subt_off[q]).astype(np.int16)
            dl = (dst_s[lo:hi][m] - b * 128).astype(np.int32)
            nm = norm_s[lo:hi][m]
            per_q.append((sl, dl, nm))
            blk_q_cnt[b, q] = len(sl)
        blk_edges.append(per_q)

    # Slot grouping: blocks sharing a slot should have near-identical per-q
    # edge counts, since seg_len[s][q] = max over the group. Self-loops
    # concentrate ~128 edges in a block's home subtable, so group by
    # (home subtable, size rank); within a slot, biggest block goes to the
    # least-loaded core (balances per-core totals).
    home = np.searchsorted(p.subt_off[1:], np.arange(n_blocks) * 128,
                           side="right")
    groups = []
    for q in range(n_subt):
        hb = np.where(home == q)[0]
        hb = hb[np.argsort(-blk_cnt[hb], kind="stable")]
        assert len(hb) % n_cores == 0, (q, len(hb))
        for i in range(0, len(hb), n_cores):
            groups.append(hb[i:i + n_cores])
    assert len(groups) == slots
    groups = _optimize_groups(blk_q_cnt, groups, home, n_cores)
    core_loads = np.zeros(n_cores, dtype=np.int64)
    core_blocks = [[-1] * slots for _ in range(n_cores)]
    for s, group in enumerate(groups):
        free = list(range(n_cores))
        group = sorted(group, key=lambda b: -int(blk_cnt[b]))
        for b in group:  # desc by size
            c = min(free, key=lambda c: core_loads[c])
            free.remove(c)
            core_loads[c] += int(blk_cnt[b])
            core_blocks[c][s] = int(b)
    p.core_blocks = core_blocks

    # lane-level segments: seg_len[s][q] = max over cores
    seg_len = np.zeros((slots, n_subt), dtype=np.int64)
    for c in range(n_cores):
        for s in range(slots):
            seg_len[s] = np.maximum(seg_len[s], blk_q_cnt[core_blocks[c][s]])
    p.seg_len = seg_len
    seg_start = np.zeros((slots, n_subt), dtype=np.int64)
    cur = np.zeros(n_subt, dtype=np.int64)
    for s in range(slots):
        seg_start[s] = cur
        cur += seg_len[s]
    p.seg_start = seg_start
    stream_len = [int(-(-int(cur[q]) // 128) * 128) for q in range(n_subt)]
    p.stream_len = stream_len
    p.n_batches = [math.ceil(sl / (gb * 128)) if sl else 0 for sl in stream_len]

    # instance enumeration (program order: slot-major, q, column)
    insts = []
    slot_inst_range = [0]
    inst_index = {}
    for s in range(slots):
        cnt = 0
        for q in range(n_subt):
            L0 = int(seg_start[s][q])
            L1 = L0 + int(seg_len[s][q])
            if L1 > L0:
                for col in range(L0 // 128, -(-L1 // 128)):
                    inst_index[(s, q, col)] = len(insts)
                    insts.append((s, q, col))
                    cnt += 1
        if cnt == 0:  # fully-empty slot across all cores (pad blocks)
            col = min(int(seg_start[s][0]) // 128,
                      max(stream_len[0] // 128 - 1, 0))
            inst_index[(s, 0, col)] = len(insts)
            insts.append((s, 0, col))
            cnt = 1
        slot_inst_range.append(len(insts))
    p.n_inst = len(insts)
    p.inst_slot = np.asarray([i[0] for i in insts], dtype=np.int64)
    p.inst_q = np.asarray([i[1] for i in insts], dtype=np.int64)
    p.inst_col = np.asarray([i[2] for i in insts], dtype=np.int64)
    p.slot_inst_range = slot_inst_range

    # per-core tables
    p.core_idx = []      # [n_cores][q] int16 wrapped [128, stream_len/16]
    p.core_dst_rel = []  # [n_cores] bf16 [128, n_inst]
    p.core_w = []        # [n_cores] bf16 [128, n_inst]
    for c in range(n_cores):
        idx_q = [np.zeros(stream_len[q], dtype=np.int16)
                 for q in range(n_subt)]
        dst_rel = np.full((128, p.n_inst), -1.0, dtype=np.float32)
        w_lane = np.zeros((128, p.n_inst), dtype=np.float32)
        for s in range(slots):
            b = core_blocks[c][s]
            for q in range(n_subt):
                sl, dl, nm = blk_edges[b][q]
                n_e = len(sl)
                if n_e == 0:
                    continue
                L0 = int(seg_start[s][q])
                idx_q[q][L0:L0 + n_e] = sl
                lanes = L0 + np.arange(n_e)
                cols = lanes // 128
                rows = lanes % 128
                iis = np.asarray([inst_index[(s, q, int(cc))] for cc in cols],
                                 dtype=np.int64)
                dst_rel[rows, iis] = dl
                w_lane[rows, iis] = nm
        idx_wrapped = []
        for q in range(n_subt):
            if stream_len[q] == 0:
                idx_wrapped.append(np.zeros((128, 1), dtype=np.int16))
                continue
            a = idx_q[q].reshape(-1, 16).T  # [16, L/16]
            idx_wrapped.append(np.tile(a, (8, 1)).copy())
        p.core_idx.append(idx_wrapped)
        p.core_dst_rel.append(dst_rel.astype(ml_dtypes.bfloat16))
        p.core_w.append(w_lane.astype(ml_dtypes.bfloat16))

    p.plan_time = time.time() - t0
    return p


# ---------------------------------------------------------------------------
# Device kernel
# ---------------------------------------------------------------------------


def build_nc(p, n_cores=None):
    d = p.d
    gb, sg, ob = p.gb, p.sg, p.ob

    nc = bacc.Bacc("TRN2", target_bir_lowering=False, debug=False,
                   num_devices=n_cores or p.n_cores)

    X = nc.dram_tensor("X", [p.n_pad, d], BF16, kind="ExternalInput")
    WT = nc.dram_tensor("WT", [d, d], BF16, kind="ExternalInput")
    iota = nc.dram_tensor("iota", [128, 128], BF16, kind="ExternalInput")
    dst_rel = nc.dram_tensor("dst_rel", [128, p.n_inst], BF16,
                             kind="ExternalInput")
    w_t = nc.dram_tensor("w_t", [128, p.n_inst], BF16, kind="ExternalInput")
    idx_t = [nc.dram_tensor(f"idx{q}", [128, max(p.stream_len[q] // 16, 1)],
                            I16, kind="ExternalInput")
             for q in range(p.n_subt)]
    out_t = nc.dram_tensor("out", [d, p.slots * 128], BF16,
                           kind="ExternalOutput")

    with tile.TileContext(nc) as tc, ExitStack() as ctx:
        cpool = ctx.enter_context(tc.tile_pool(name="consts", bufs=1))
        WT_sb = cpool.tile([d, d], BF16)
        nc.sync.dma_start(WT_sb[:], WT.ap()[:, :])
        iota_sb = cpool.tile([128, 128], BF16)
        nc.sync.dma_start(iota_sb[:], iota.ap()[:, :])
        dstrel_lo = cpool.tile([128, p.n_inst], BF16)
        nc.sync.dma_start(dstrel_lo[:], dst_rel.ap()[:, :])
        w_lo = cpool.tile([128, p.n_inst], BF16)
        nc.sync.dma_start(w_lo[:], w_t.ap()[:, :])
        dstrel_sb = cpool.tile([128, p.n_inst], F32)
        nc.vector.tensor_scalar_mul(dstrel_sb[:], dstrel_lo[:], 1.0)
        w_sb = cpool.tile([128, p.n_inst], F32)
        nc.vector.tensor_scalar_mul(w_sb[:], w_lo[:], 1.0)
        idx_sb = []
        for q in range(p.n_subt):
            t = cpool.tile([128, idx_t[q].shape[1]], I16, name=f"idxsb{q}")
            nc.sync.dma_start(t[:], idx_t[q].ap()[:, :])
            idx_sb.append(t)

        gpools = [ctx.enter_context(tc.tile_pool(name=f"g{q}", bufs=2))
                  for q in range(p.n_subt)]
        stpool = ctx.enter_context(tc.tile_pool(name="st", bufs=6))
        aggpool = ctx.enter_context(tc.tile_pool(name="agg", bufs=3))
        opool = ctx.enter_context(tc.tile_pool(name="ostage", bufs=2))
        pa = ctx.enter_context(tc.tile_pool(name="pa", bufs=4, space="PSUM"))
        pb = ctx.enter_context(tc.tile_pool(name="pb", bufs=4, space="PSUM"))

        gtiles = {}

        def get_gtile(q, i):
            if (q, i) not in gtiles:
                lanes = min(gb * 128, p.stream_len[q] - i * gb * 128)
                gt = gpools[q].tile([128, gb * d], BF16, name=f"gt{q}")
                nc.gpsimd.dma_gather(
                    out_ap=gt[:, 0:(lanes // 128) * d].rearrange(
                        "p (j d) -> p j d", d=d),
                    in_ap=X.ap()[int(p.subt_off[q]):int(p.subt_off[q + 1]), :],
                    idxs_ap=idx_sb[q][:, i * gb * 8: i * gb * 8 + lanes // 16],
                    num_idxs=lanes,
                    num_idxs_reg=lanes,
                    elem_size=d,
                    single_packet=False,
                )
                gtiles[(q, i)] = gt
            return gtiles[(q, i)]

        ostage = None
        for s in range(p.slots):
            ps_a = pa.tile([128, d], F32, name="pa_t", space="PSUM")
            i0, i1 = p.slot_inst_range[s], p.slot_inst_range[s + 1]
            for ii in range(i0, i1):
                # weighted one-hot: st[r, j] = (iota[j] == dst_rel[r]) * w[r]
                st_t = stpool.tile([128, 128], BF16, name="st_t")
                nc.vector.tensor_scalar(
                    out=st_t[:], in0=iota_sb[:],
                    scalar1=dstrel_sb[:, ii:ii + 1],
                    scalar2=w_sb[:, ii:ii + 1],
                    op0=mybir.AluOpType.is_equal,
                    op1=mybir.AluOpType.mult)
                q = int(p.inst_q[ii])
                c = int(p.inst_col[ii])
                gt = get_gtile(q, c // gb)
                o = (c % gb) * d
                nc.tensor.matmul(out=ps_a[:], lhsT=gt[:, o:o + d],
                                 rhs=st_t[:],
                                 start=(ii == i0), stop=(ii == i1 - 1))
            agg = aggpool.tile([128, d], BF16, name="agg_t")
            nc.scalar.activation(agg[:], ps_a[:],
                                 mybir.ActivationFunctionType.Copy)
            ps_b = pb.tile([128, d], F32, name="pb_t", space="PSUM")
            nc.tensor.matmul(out=ps_b[:], lhsT=WT_sb[:], rhs=agg[:],
                             start=True, stop=True)
            if s % ob == 0:
                ostage = opool.tile([128, ob * d], BF16, name="ostage")
            ocol = (s % ob) * d
            nc.scalar.activation(ostage[:, ocol:ocol + d], ps_b[:],
                                 mybir.ActivationFunctionType.Copy)
            if s % ob == ob - 1 or s == p.slots - 1:
                s0 = s // ob * ob
                nsw = s - s0 + 1
                nc.sync.dma_start(
                    out=out_t.ap()[:, s0 * 128:(s0 + nsw) * 128],
                    in_=ostage[:, 0:nsw * d])

    nc.compile()
    return nc


# ---------------------------------------------------------------------------
# Orchestration
# ---------------------------------------------------------------------------


def make_inputs(p, x, W):
    d = p.d
    Xb = np.zeros((p.n_pad, d), dtype=ml_dtypes.bfloat16)
    Xb[:p.n_nodes] = np.asarray(x, dtype=np.float32).astype(ml_dtypes.bfloat16)
    WT = np.ascontiguousarray(np.asarray(W, dtype=np.float32).T).astype(
        ml_dtypes.bfloat16)
    iota = np.broadcast_to(np.arange(128, dtype=np.float32),
                           (128, 128)).astype(ml_dtypes.bfloat16).copy()
    common = {"X": Xb, "WT": WT, "iota": iota}
    in_maps = []
    for c in range(p.n_cores):
        m = dict(common)
        m["dst_rel"] = p.core_dst_rel[c]
        m["w_t"] = p.core_w[c]
        for q in range(p.n_subt):
            m[f"idx{q}"] = p.core_idx[c][q]
        in_maps.append(m)
    return in_maps


def assemble_output(p, results, b):
    out = np.zeros((p.n_nodes, p.d), dtype=np.float32)
    for c in range(p.n_cores):
        oc = np.asarray(results[c]["out"], dtype=np.float32)  # [d, slots*128]
        for s, blk in enumerate(p.core_blocks[c]):
            lo = blk * 128
            if lo >= p.n_nodes:
                continue
            hi = min(lo + 128, p.n_nodes)
            out[lo:hi] = oc[:, s * 128: s * 128 + (hi - lo)].T
    # bias epilogue: out[v] += rdeg[v] * sraw[v] * b
    coef = (p.rdeg[:p.n_nodes] * p.sraw[:p.n_nodes]).astype(np.float32)
    out += coef[:, None] * np.asarray(b, dtype=np.float32)[None, :]
    return out


def gcn_forward(x, edge_index, W, b, n_cores=8, trace=False, **plan_kw):
    n = x.shape[0]
    src = np.asarray(edge_index[0])
    dst = np.asarray(edge_index[1])
    loop = np.arange(n, dtype=src.dtype)
    src_all = np.concatenate([src, loop])
    dst_all = np.concatenate([dst, loop])
    p = build_plan(src_all, dst_all, n, n_cores, d=W.shape[0], **plan_kw)
    nc = build_nc(p)
    in_maps = make_inputs(p, x, W)
    res = run_bass_kernel_spmd(nc, in_maps, core_ids=list(range(n_cores)),
                               trace=trace)
    out = assemble_output(p, [r for r in res.results], b)
    return out, p, res


def kernel(x, edge_index, W, b):
    """GCN layer forward on 8 trn2 NeuronCores. Inputs as in setup_inputs()."""
    x = np.asarray(x, dtype=np.float32)
    edge_index = np.asarray(edge_index)
    W = np.asarray(W, dtype=np.float32)
    b = np.asarray(b, dtype=np.float32)
    out, _p, _res = gcn_forward(x, edge_index, W, b, n_cores=N_CORES)
    return out.astype(np.float32)


# revision 21
# speedup vs baseline: 1.9050x; 1.0582x over previous
"""GCN layer (out = D^-1/2 (A+I) D^-1/2 (x W^T + b)) on 8 trn2 NeuronCores.

Strategy (v2 — direct-x aggregation, no intermediate table):
  out[dst] = W @ (sum_e norm_e * x[src_e]) + (sum_e norm_e) * b
  with norm_e = rdeg[src]*rdeg[dst].

  - Host: append self-loops, sort edges by dst, partition dst-blocks (128
    nodes) across 8 cores (LPT), build a core-invariant static schedule:
    per (slot, subtable) segments padded at LANE level to the max across
    cores (uniform SPMD program; per-core idx/weight tables).
  - Device: gather raw x rows (bf16, 256B) per edge lane straight from the
    uploaded input table (4 int16-indexable subtable views); accumulate
    aggT[din, dst] = sum_lane x_lane[din] * S[lane, dst] on the PE, where
    S = weighted one-hot (weight = norm_e, built by DVE is_equal * w);
    then one W matmul per dst block; bias term applied on host
    (out += rdeg*sraw*b, with sraw = sum_e rdeg[src]).

  vs v1: removes the whole P1 phase (x stream + T-table write, ~215us of
  modeled DMA) and ~18% gather lane fragmentation.
"""

import math
import time
from contextlib import ExitStack

import ml_dtypes
import numpy as np

import concourse.bass as bass
import concourse.tile as tile
from concourse import bacc, mybir
from concourse.bass_utils import run_bass_kernel_spmd

F32 = mybir.dt.float32
BF16 = mybir.dt.bfloat16
I16 = mybir.dt.int16

N_NODES = 100000
N_EDGES = 1600000
IN_CH = 128
OUT_CH = 128
N_CORES = 8

# ---------------------------------------------------------------------------
# Host-side planning
# ---------------------------------------------------------------------------


class Plan:
    pass


def _optimize_groups(blk_q_cnt, groups, home, n_cores, rounds=60,
                     batch=16384, seed=7):
    """Vectorized local search: swap blocks between slot-groups of the same
    home class to reduce sum over groups of sum_q max (padded lane count)."""
    rng = np.random.default_rng(seed)
    gmat = np.stack([np.asarray(g) for g in groups])          # [G, C]
    G, C = gmat.shape
    ghome = home[gmat[:, 0]]
    obj = blk_q_cnt[gmat].max(axis=1).sum(axis=1)             # [G]
    byhome = {}
    for i in range(G):
        byhome.setdefault(int(ghome[i]), []).append(i)
    classes = [np.asarray(v) for v in byhome.values() if len(v) > 1]
    if not classes:
        return [gmat[i] for i in range(G)]
    for _ in range(rounds):
        # class-matched random group pairs
        cls = [c[rng.integers(0, len(c), size=(batch, 2))] for c in classes]
        pij = np.concatenate(cls, axis=0)
        keep = pij[:, 0] != pij[:, 1]
        pij = pij[keep]
        K = len(pij)
        ab = rng.integers(0, C, size=(K, 2))
        gi = gmat[pij[:, 0]].copy()                           # [K, C]
        gj = gmat[pij[:, 1]].copy()
        bi = gj[np.arange(K), ab[:, 1]].copy()
        gj[np.arange(K), ab[:, 1]] = gi[np.arange(K), ab[:, 0]]
        gi[np.arange(K), ab[:, 0]] = bi
        oi = blk_q_cnt[gi].max(axis=1).sum(axis=1)
        oj = blk_q_cnt[gj].max(axis=1).sum(axis=1)
        delta = (oi + oj) - (obj[pij[:, 0]] + obj[pij[:, 1]])
        good = np.where(delta < 0)[0]
        if len(good) == 0:
            continue
        good = good[np.argsort(delta[good])]
        used = np.zeros(G, dtype=bool)
        for k in good:
            i, j = pij[k]
            if used[i] or used[j]:
                continue
            used[i] = used[j] = True
            gmat[i] = gi[k]
            gmat[j] = gj[k]
            obj[i] = oi[k]
            obj[j] = oj[k]
    return [gmat[i] for i in range(G)]


def build_plan(src_all, dst_all, n_nodes, n_cores, d=128, gb=24, sg=8, ob=8,
               subt_cap=32768):
    """src_all/dst_all: edge endpoints INCLUDING self loops."""
    t0 = time.time()
    p = Plan()
    p.d = d
    p.gb = gb
    p.sg = sg
    p.ob = ob
    p.n_nodes = n_nodes
    p.n_cores = n_cores

    n_blocks = math.ceil(n_nodes / 128)
    slots = math.ceil(n_blocks / n_cores)
    n_blocks = slots * n_cores
    p.slots = slots
    p.n_blocks = n_blocks
    p.n_pad = n_blocks * 128

    # subtables over node id range (int16 gather index limit)
    n_subt = math.ceil(p.n_pad / subt_cap)
    off = [min(q * subt_cap, p.n_pad) for q in range(n_subt + 1)]
    p.subt_off = np.asarray(off)
    p.n_subt = n_subt

    src_all = np.asarray(src_all, dtype=np.int64)
    dst_all = np.asarray(dst_all, dtype=np.int64)

    # degrees / norms (host; exactly mirrors the reference formula)
    deg = np.bincount(dst_all, minlength=p.n_pad).astype(np.float64)
    deg[n_nodes:] = 1.0
    deg[deg == 0] = 1.0
    rdeg = 1.0 / np.sqrt(deg)
    p.rdeg = rdeg.astype(np.float32)
    norm_all = (rdeg[src_all] * rdeg[dst_all]).astype(np.float32)
    # host bias epilogue: out[v] += rdeg[v] * sraw[v] * b
    sraw = np.zeros(p.n_pad, dtype=np.float64)
    np.add.at(sraw, dst_all, rdeg[src_all])
    p.sraw = sraw.astype(np.float32)

    # sort edges by dst
    order = np.argsort(dst_all, kind="stable")
    dst_s = dst_all[order]
    src_s = src_all[order]
    norm_s = norm_all[order]
    blk_ptr = np.searchsorted(dst_s, np.arange(0, p.n_pad + 1, 128))
    blk_cnt = blk_ptr[1:] - blk_ptr[:-1]

    # per (block, q) edge sublists
    subt_of_src = np.searchsorted(p.subt_off[1:], src_s, side="right")
    blk_edges = []  # [block][q] -> (src_local, dst_local, norm)
    blk_q_cnt = np.zeros((n_blocks, n_subt), dtype=np.int64)
    for b in range(n_blocks):
        lo, hi = blk_ptr[b], blk_ptr[b + 1]
        qs = subt_of_src[lo:hi]
        per_q = []
        for q in range(n_subt):
            m = qs == q
            sl = (src_s[lo:hi][m] - p.subt_off[q]).astype(np.int16)
            dl = (dst_s[lo:hi][m] - b * 128).astype(np.int32)
            nm = norm_s[lo:hi][m]
            per_q.append((sl, dl, nm))
            blk_q_cnt[b, q] = len(sl)
        blk_edges.append(per_q)

    # Slot grouping: blocks sharing a slot should have near-identical per-q
    # edge counts, since seg_len[s][q] = max over the group. Self-loops
    # concentrate ~128 edges in a block's home subtable, so group by
    # (home subtable, size rank); within a slot, biggest block goes to the
    # least-loaded core (balances per-core totals).
    home = np.searchsorted(p.subt_off[1:], np.arange(n_blocks) * 128,
                           side="right")
    groups = []
    for q in range(n_subt):
        hb = np.where(home == q)[0]
        hb = hb[np.argsort(-blk_cnt[hb], kind="stable")]
        assert len(hb) % n_cores == 0, (q, len(hb))
        for i in range(0, len(hb), n_cores):
            groups.append(hb[i:i + n_cores])
    assert len(groups) == slots
    groups = _optimize_groups(blk_q_cnt, groups, home, n_cores)
    core_loads = np.zeros(n_cores, dtype=np.int64)
    core_blocks = [[-1] * slots for _ in range(n_cores)]
    for s, group in enumerate(groups):
        free = list(range(n_cores))
        group = sorted(group, key=lambda b: -int(blk_cnt[b]))
        for b in group:  # desc by size
            c = min(free, key=lambda c: core_loads[c])
            free.remove(c)
            core_loads[c] += int(blk_cnt[b])
            core_blocks[c][s] = int(b)
    p.core_blocks = core_blocks

    # lane-level segments: seg_len[s][q] = max over cores
    seg_len = np.zeros((slots, n_subt), dtype=np.int64)
    for c in range(n_cores):
        for s in range(slots):
            seg_len[s] = np.maximum(seg_len[s], blk_q_cnt[core_blocks[c][s]])
    p.seg_len = seg_len
    seg_start = np.zeros((slots, n_subt), dtype=np.int64)
    cur = np.zeros(n_subt, dtype=np.int64)
    for s in range(slots):
        seg_start[s] = cur
        cur += seg_len[s]
    p.seg_start = seg_start
    stream_len = [int(-(-int(cur[q]) // 128) * 128) for q in range(n_subt)]
    p.stream_len = stream_len

    # gather batch boundaries per stream, in columns. Full gb-col batches,
    # but ramp the tail down so the post-last-gather matmul drain is short.
    p.batch_bounds = []  # [q] -> list of (col0, ncols)
    for q in range(n_subt):
        cols = stream_len[q] // 128
        bounds = []
        c0 = 0
        while cols - c0 > gb:
            bounds.append((c0, gb))
            c0 += gb
        r = cols - c0
        while r > 6:
            t = max(6, -(-r // 2))
            bounds.append((c0, t))
            c0 += t
            r -= t
        if r > 0:
            bounds.append((c0, r))
        p.batch_bounds.append(bounds)

    # instance enumeration (program order: slot-major, q, column)
    insts = []
    slot_inst_range = [0]
    inst_index = {}
    for s in range(slots):
        cnt = 0
        for q in range(n_subt):
            L0 = int(seg_start[s][q])
            L1 = L0 + int(seg_len[s][q])
            if L1 > L0:
                for col in range(L0 // 128, -(-L1 // 128)):
                    inst_index[(s, q, col)] = len(insts)
                    insts.append((s, q, col))
                    cnt += 1
        if cnt == 0:  # fully-empty slot across all cores (pad blocks)
            col = min(int(seg_start[s][0]) // 128,
                      max(stream_len[0] // 128 - 1, 0))
            inst_index[(s, 0, col)] = len(insts)
            insts.append((s, 0, col))
            cnt = 1
        slot_inst_range.append(len(insts))
    p.n_inst = len(insts)
    p.inst_slot = np.asarray([i[0] for i in insts], dtype=np.int64)
    p.inst_q = np.asarray([i[1] for i in insts], dtype=np.int64)
    p.inst_col = np.asarray([i[2] for i in insts], dtype=np.int64)
    p.slot_inst_range = slot_inst_range

    # per-core tables
    p.core_idx = []      # [n_cores][q] int16 wrapped [128, stream_len/16]
    p.core_dst_rel = []  # [n_cores] bf16 [128, n_inst]
    p.core_w = []        # [n_cores] bf16 [128, n_inst]
    for c in range(n_cores):
        idx_q = [np.zeros(stream_len[q], dtype=np.int16)
                 for q in range(n_subt)]
        dst_rel = np.full((128, p.n_inst), -1.0, dtype=np.float32)
        w_lane = np.zeros((128, p.n_inst), dtype=np.float32)
        for s in range(slots):
            b = core_blocks[c][s]
            for q in range(n_subt):
                sl, dl, nm = blk_edges[b][q]
                n_e = len(sl)
                if n_e == 0:
                    continue
                L0 = int(seg_start[s][q])
                idx_q[q][L0:L0 + n_e] = sl
                lanes = L0 + np.arange(n_e)
                cols = lanes // 128
                rows = lanes % 128
                iis = np.asarray([inst_index[(s, q, int(cc))] for cc in cols],
                                 dtype=np.int64)
                dst_rel[rows, iis] = dl
                w_lane[rows, iis] = nm
        idx_wrapped = []
        for q in range(n_subt):
            if stream_len[q] == 0:
                idx_wrapped.append(np.zeros((128, 1), dtype=np.int16))
                continue
            a = idx_q[q].reshape(-1, 16).T  # [16, L/16]
            idx_wrapped.append(np.tile(a, (8, 1)).copy())
        p.core_idx.append(idx_wrapped)
        p.core_dst_rel.append(dst_rel.astype(ml_dtypes.bfloat16))
        p.core_w.append(w_lane.astype(ml_dtypes.bfloat16))

    p.plan_time = time.time() - t0
    return p


# ---------------------------------------------------------------------------
# Device kernel
# ---------------------------------------------------------------------------


def build_nc(p, n_cores=None):
    d = p.d
    gb, sg, ob = p.gb, p.sg, p.ob

    nc = bacc.Bacc("TRN2", target_bir_lowering=False, debug=False,
                   num_devices=n_cores or p.n_cores)

    X = nc.dram_tensor("X", [p.n_pad, d], BF16, kind="ExternalInput")
    WT = nc.dram_tensor("WT", [d, d], BF16, kind="ExternalInput")
    iota = nc.dram_tensor("iota", [128, 128], BF16, kind="ExternalInput")
    dst_rel = nc.dram_tensor("dst_rel", [128, p.n_inst], BF16,
                             kind="ExternalInput")
    w_t = nc.dram_tensor("w_t", [128, p.n_inst], BF16, kind="ExternalInput")
    idx_t = [nc.dram_tensor(f"idx{q}", [128, max(p.stream_len[q] // 16, 1)],
                            I16, kind="ExternalInput")
             for q in range(p.n_subt)]
    out_t = nc.dram_tensor("out", [d, p.slots * 128], BF16,
                           kind="ExternalOutput")

    with tile.TileContext(nc) as tc, ExitStack() as ctx:
        cpool = ctx.enter_context(tc.tile_pool(name="consts", bufs=1))
        WT_sb = cpool.tile([d, d], BF16)
        nc.sync.dma_start(WT_sb[:], WT.ap()[:, :])
        iota_sb = cpool.tile([128, 128], BF16)
        nc.sync.dma_start(iota_sb[:], iota.ap()[:, :])
        dstrel_lo = cpool.tile([128, p.n_inst], BF16)
        nc.sync.dma_start(dstrel_lo[:], dst_rel.ap()[:, :])
        w_lo = cpool.tile([128, p.n_inst], BF16)
        nc.sync.dma_start(w_lo[:], w_t.ap()[:, :])
        dstrel_sb = cpool.tile([128, p.n_inst], F32)
        nc.vector.tensor_scalar_mul(dstrel_sb[:], dstrel_lo[:], 1.0)
        w_sb = cpool.tile([128, p.n_inst], F32)
        nc.vector.tensor_scalar_mul(w_sb[:], w_lo[:], 1.0)
        idx_sb = []
        for q in range(p.n_subt):
            t = cpool.tile([128, idx_t[q].shape[1]], I16, name=f"idxsb{q}")
            nc.sync.dma_start(t[:], idx_t[q].ap()[:, :])
            idx_sb.append(t)

        gpools = [ctx.enter_context(tc.tile_pool(name=f"g{q}", bufs=2))
                  for q in range(p.n_subt)]
        stpool = ctx.enter_context(tc.tile_pool(name="st", bufs=6))
        aggpool = ctx.enter_context(tc.tile_pool(name="agg", bufs=3))
        opool = ctx.enter_context(tc.tile_pool(name="ostage", bufs=2))
        pa = ctx.enter_context(tc.tile_pool(name="pa", bufs=4, space="PSUM"))
        pb = ctx.enter_context(tc.tile_pool(name="pb", bufs=4, space="PSUM"))

        gtiles = {}
        colbatch = []
        for q in range(p.n_subt):
            cb = np.zeros(max(p.stream_len[q] // 128, 1), dtype=np.int64)
            for bi, (c0, ncols) in enumerate(p.batch_bounds[q]):
                cb[c0:c0 + ncols] = bi
            colbatch.append(cb)

        def get_gtile(q, i):
            if (q, i) not in gtiles:
                c0, ncols = p.batch_bounds[q][i]
                lanes = ncols * 128
                gt = gpools[q].tile([128, gb * d], BF16, name=f"gt{q}")
                nc.gpsimd.dma_gather(
                    out_ap=gt[:, 0:ncols * d].rearrange(
                        "p (j d) -> p j d", d=d),
                    in_ap=X.ap()[int(p.subt_off[q]):int(p.subt_off[q + 1]), :],
                    idxs_ap=idx_sb[q][:, c0 * 8: c0 * 8 + lanes // 16],
                    num_idxs=lanes,
                    num_idxs_reg=lanes,
                    elem_size=d,
                    single_packet=False,
                )
                gtiles[(q, i)] = gt
            return gtiles[(q, i)]

        ostage = None
        for s in range(p.slots):
            ps_a = pa.tile([128, d], F32, name="pa_t", space="PSUM")
            i0, i1 = p.slot_inst_range[s], p.slot_inst_range[s + 1]
            for ii in range(i0, i1):
                # weighted one-hot: st[r, j] = (iota[j] == dst_rel[r]) * w[r]
                st_t = stpool.tile([128, 128], BF16, name="st_t")
                nc.vector.tensor_scalar(
                    out=st_t[:], in0=iota_sb[:],
                    scalar1=dstrel_sb[:, ii:ii + 1],
                    scalar2=w_sb[:, ii:ii + 1],
                    op0=mybir.AluOpType.is_equal,
                    op1=mybir.AluOpType.mult)
                q = int(p.inst_q[ii])
                c = int(p.inst_col[ii])
                bi = int(colbatch[q][c])
                gt = get_gtile(q, bi)
                o = (c - p.batch_bounds[q][bi][0]) * d
                nc.tensor.matmul(out=ps_a[:], lhsT=gt[:, o:o + d],
                                 rhs=st_t[:],
                                 start=(ii == i0), stop=(ii == i1 - 1))
            agg = aggpool.tile([128, d], BF16, name="agg_t")
            if s % 2 == 0:
                nc.scalar.activation(agg[:], ps_a[:],
                                     mybir.ActivationFunctionType.Copy)
            else:
                nc.vector.tensor_scalar_mul(agg[:], ps_a[:], 1.0)
            ps_b = pb.tile([128, d], F32, name="pb_t", space="PSUM")
            nc.tensor.matmul(out=ps_b[:], lhsT=WT_sb[:], rhs=agg[:],
                             start=True, stop=True)
            if s % ob == 0:
                ostage = opool.tile([128, ob * d], BF16, name="ostage")
            ocol = (s % ob) * d
            if s % 2 == 0:
                nc.vector.tensor_scalar_mul(ostage[:, ocol:ocol + d],
                                            ps_b[:], 1.0)
            else:
                nc.scalar.activation(ostage[:, ocol:ocol + d], ps_b[:],
                                     mybir.ActivationFunctionType.Copy)
            if s % ob == ob - 1 or s == p.slots - 1:
                s0 = s // ob * ob
                nsw = s - s0 + 1
                nc.sync.dma_start(
                    out=out_t.ap()[:, s0 * 128:(s0 + nsw) * 128],
                    in_=ostage[:, 0:nsw * d])

    nc.compile()
    return nc


# ---------------------------------------------------------------------------
# Orchestration
# ---------------------------------------------------------------------------


def make_inputs(p, x, W):
    d = p.d
    Xb = np.zeros((p.n_pad, d), dtype=ml_dtypes.bfloat16)
    Xb[:p.n_nodes] = np.asarray(x, dtype=np.float32).astype(ml_dtypes.bfloat16)
    WT = np.ascontiguousarray(np.asarray(W, dtype=np.float32).T).astype(
        ml_dtypes.bfloat16)
    iota = np.broadcast_to(np.arange(128, dtype=np.float32),
                           (128, 128)).astype(ml_dtypes.bfloat16).copy()
    common = {"X": Xb, "WT": WT, "iota": iota}
    in_maps = []
    for c in range(p.n_cores):
        m = dict(common)
        m["dst_rel"] = p.core_dst_rel[c]
        m["w_t"] = p.core_w[c]
        for q in range(p.n_subt):
            m[f"idx{q}"] = p.core_idx[c][q]
        in_maps.append(m)
    return in_maps


def assemble_output(p, results, b):
    out = np.zeros((p.n_nodes, p.d), dtype=np.float32)
    for c in range(p.n_cores):
        oc = np.asarray(results[c]["out"], dtype=np.float32)  # [d, slots*128]
        for s, blk in enumerate(p.core_blocks[c]):
            lo = blk * 128
            if lo >= p.n_nodes:
                continue
            hi = min(lo + 128, p.n_nodes)
            out[lo:hi] = oc[:, s * 128: s * 128 + (hi - lo)].T
    # bias epilogue: out[v] += rdeg[v] * sraw[v] * b
    coef = (p.rdeg[:p.n_nodes] * p.sraw[:p.n_nodes]).astype(np.float32)
    out += coef[:, None] * np.asarray(b, dtype=np.float32)[None, :]
    return out


def gcn_forward(x, edge_index, W, b, n_cores=8, trace=False, **plan_kw):
    n = x.shape[0]
    src = np.asarray(edge_index[0])
    dst = np.asarray(edge_index[1])
    loop = np.arange(n, dtype=src.dtype)
    src_all = np.concatenate([src, loop])
    dst_all = np.concatenate([dst, loop])
    p = build_plan(src_all, dst_all, n, n_cores, d=W.shape[0], **plan_kw)
    nc = build_nc(p)
    in_maps = make_inputs(p, x, W)
    res = run_bass_kernel_spmd(nc, in_maps, core_ids=list(range(n_cores)),
                               trace=trace)
    out = assemble_output(p, [r for r in res.results], b)
    return out, p, res


def kernel(x, edge_index, W, b):
    """GCN layer forward on 8 trn2 NeuronCores. Inputs as in setup_inputs()."""
    x = np.asarray(x, dtype=np.float32)
    edge_index = np.asarray(edge_index)
    W = np.asarray(W, dtype=np.float32)
    b = np.asarray(b, dtype=np.float32)
    out, _p, _res = gcn_forward(x, edge_index, W, b, n_cores=N_CORES)
    return out.astype(np.float32)


# revision 32
# speedup vs baseline: 1.9199x; 1.0078x over previous
"""GCN layer (out = D^-1/2 (A+I) D^-1/2 (x W^T + b)) on 8 trn2 NeuronCores.

Strategy (v2 — direct-x aggregation, no intermediate table):
  out[dst] = W @ (sum_e norm_e * x[src_e]) + (sum_e norm_e) * b
  with norm_e = rdeg[src]*rdeg[dst].

  - Host: append self-loops, sort edges by dst, partition dst-blocks (128
    nodes) across 8 cores (LPT), build a core-invariant static schedule:
    per (slot, subtable) segments padded at LANE level to the max across
    cores (uniform SPMD program; per-core idx/weight tables).
  - Device: gather raw x rows (bf16, 256B) per edge lane straight from the
    uploaded input table (4 int16-indexable subtable views); accumulate
    aggT[din, dst] = sum_lane x_lane[din] * S[lane, dst] on the PE, where
    S = weighted one-hot (weight = norm_e, built by DVE is_equal * w);
    then one W matmul per dst block; bias term applied on host
    (out += rdeg*sraw*b, with sraw = sum_e rdeg[src]).

  vs v1: removes the whole P1 phase (x stream + T-table write, ~215us of
  modeled DMA) and ~18% gather lane fragmentation.
"""

import math
import time
from contextlib import ExitStack

import ml_dtypes
import numpy as np

import concourse.bass as bass
import concourse.tile as tile
from concourse import bacc, mybir
from concourse.bass_utils import run_bass_kernel_spmd

F32 = mybir.dt.float32
BF16 = mybir.dt.bfloat16
I16 = mybir.dt.int16

N_NODES = 100000
N_EDGES = 1600000
IN_CH = 128
OUT_CH = 128
N_CORES = 8

# ---------------------------------------------------------------------------
# Host-side planning
# ---------------------------------------------------------------------------


class Plan:
    pass


def _optimize_groups(blk_q_cnt, groups, home, n_cores, rounds=60,
                     batch=16384, seed=7):
    """Vectorized local search: swap blocks between slot-groups of the same
    home class to reduce sum over groups of sum_q max (padded lane count)."""
    rng = np.random.default_rng(seed)
    gmat = np.stack([np.asarray(g) for g in groups])          # [G, C]
    G, C = gmat.shape
    ghome = home[gmat[:, 0]]
    obj = blk_q_cnt[gmat].max(axis=1).sum(axis=1)             # [G]
    byhome = {}
    for i in range(G):
        byhome.setdefault(int(ghome[i]), []).append(i)
    classes = [np.asarray(v) for v in byhome.values() if len(v) > 1]
    if not classes:
        return [gmat[i] for i in range(G)]
    for _ in range(rounds):
        # class-matched random group pairs
        cls = [c[rng.integers(0, len(c), size=(batch, 2))] for c in classes]
        pij = np.concatenate(cls, axis=0)
        keep = pij[:, 0] != pij[:, 1]
        pij = pij[keep]
        K = len(pij)
        ab = rng.integers(0, C, size=(K, 2))
        gi = gmat[pij[:, 0]].copy()                           # [K, C]
        gj = gmat[pij[:, 1]].copy()
        bi = gj[np.arange(K), ab[:, 1]].copy()
        gj[np.arange(K), ab[:, 1]] = gi[np.arange(K), ab[:, 0]]
        gi[np.arange(K), ab[:, 0]] = bi
        oi = blk_q_cnt[gi].max(axis=1).sum(axis=1)
        oj = blk_q_cnt[gj].max(axis=1).sum(axis=1)
        delta = (oi + oj) - (obj[pij[:, 0]] + obj[pij[:, 1]])
        good = np.where(delta < 0)[0]
        if len(good) == 0:
            continue
        good = good[np.argsort(delta[good])]
        used = np.zeros(G, dtype=bool)
        for k in good:
            i, j = pij[k]
            if used[i] or used[j]:
                continue
            used[i] = used[j] = True
            gmat[i] = gi[k]
            gmat[j] = gj[k]
            obj[i] = oi[k]
            obj[j] = oj[k]
    return [gmat[i] for i in range(G)]


def build_plan(src, dst, n_nodes, n_cores, d=128, gb=16, sg=8, ob=8,
               subt_cap=32768):
    """src/dst: REAL edge endpoints (self loops handled separately)."""
    t0 = time.time()
    p = Plan()
    p.d = d
    p.gb = gb
    p.sg = sg
    p.ob = ob
    p.n_nodes = n_nodes
    p.n_cores = n_cores

    n_blocks = math.ceil(n_nodes / 128)
    slots = math.ceil(n_blocks / n_cores)
    n_blocks = slots * n_cores
    p.slots = slots
    p.n_blocks = n_blocks
    p.n_pad = n_blocks * 128

    # subtables over node id range (int16 gather index limit)
    n_subt = math.ceil(p.n_pad / subt_cap)
    off = [min(q * subt_cap, p.n_pad) for q in range(n_subt + 1)]
    p.subt_off = np.asarray(off)
    p.n_subt = n_subt

    src = np.asarray(src, dtype=np.int64)
    dst = np.asarray(dst, dtype=np.int64)
    loop = np.arange(n_nodes, dtype=np.int64)

    # degrees / norms (host; exactly mirrors the reference formula,
    # self-loops included in the degree)
    deg = np.bincount(np.concatenate([dst, loop]),
                      minlength=p.n_pad).astype(np.float64)
    deg[n_nodes:] = 1.0
    deg[deg == 0] = 1.0
    rdeg = 1.0 / np.sqrt(deg)
    p.rdeg = rdeg.astype(np.float32)
    norm_real = (rdeg[src] * rdeg[dst]).astype(np.float32)
    # host bias epilogue: out[v] += rdeg[v] * sraw[v] * b  (incl. self loop)
    sraw = np.zeros(p.n_pad, dtype=np.float64)
    np.add.at(sraw, dst, rdeg[src])
    sraw[:n_nodes] += rdeg[:n_nodes]
    p.sraw = sraw.astype(np.float32)
    # self-loop weight per node (x[v] contribution), 0 for pad nodes
    wself = (rdeg * rdeg).astype(np.float32)
    wself[n_nodes:] = 0.0
    p.wself = wself

    # sort REAL edges by dst
    order = np.argsort(dst, kind="stable")
    dst_s = dst[order]
    src_s = src[order]
    norm_s = norm_real[order]
    blk_ptr = np.searchsorted(dst_s, np.arange(0, p.n_pad + 1, 128))
    blk_cnt = blk_ptr[1:] - blk_ptr[:-1]

    # per (block, q) edge sublists
    subt_of_src = np.searchsorted(p.subt_off[1:], src_s, side="right")
    blk_edges = []  # [block][q] -> (src_local, dst_local, norm)
    blk_q_cnt = np.zeros((n_blocks, n_subt), dtype=np.int64)
    for b in range(n_blocks):
        lo, hi = blk_ptr[b], blk_ptr[b + 1]
        qs = subt_of_src[lo:hi]
        per_q = []
        for q in range(n_subt):
            m = qs == q
            sl = (src_s[lo:hi][m] - p.subt_off[q]).astype(np.int16)
            dl = (dst_s[lo:hi][m] - b * 128).astype(np.int32)
            nm = norm_s[lo:hi][m]
            per_q.append((sl, dl, nm))
            blk_q_cnt[b, q] = len(sl)
        blk_edges.append(per_q)

    # Slot grouping: blocks sharing a slot should have near-identical per-q
    # edge counts, since seg_len[s][q] = max over the group. Self-loops
    # concentrate ~128 edges in a block's home subtable, so group by
    # (home subtable, size rank); within a slot, biggest block goes to the
    # least-loaded core (balances per-core totals).
    home = np.searchsorted(p.subt_off[1:], np.arange(n_blocks) * 128,
                           side="right")
    groups = []
    for q in range(n_subt):
        hb = np.where(home == q)[0]
        hb = hb[np.argsort(-blk_cnt[hb], kind="stable")]
        assert len(hb) % n_cores == 0, (q, len(hb))
        for i in range(0, len(hb), n_cores):
            groups.append(hb[i:i + n_cores])
    assert len(groups) == slots
    groups = _optimize_groups(blk_q_cnt, groups, home, n_cores)
    core_loads = np.zeros(n_cores, dtype=np.int64)
    core_blocks = [[-1] * slots for _ in range(n_cores)]
    for s, group in enumerate(groups):
        free = list(range(n_cores))
        group = sorted(group, key=lambda b: -int(blk_cnt[b]))
        for b in group:  # desc by size
            c = min(free, key=lambda c: core_loads[c])
            free.remove(c)
            core_loads[c] += int(blk_cnt[b])
            core_blocks[c][s] = int(b)
    p.core_blocks = core_blocks

    # Streams 0..n_subt-1: per-edge gathers (elem = d, one idx per edge).
    # Streams n_subt..2*n_subt-1: self-loop pair gathers (elem = 2d, one idx
    # per CONSECUTIVE NODE PAIR of the slot's own block; 64 lanes per slot,
    # in the block's home subtable). 512B descriptors avoid the <512B DMA
    # penalty, halving the per-node self-loop gather cost.
    n_streams = 2 * n_subt
    p.n_streams = n_streams
    group_home = np.zeros(slots, dtype=np.int64)
    for s in range(slots):
        group_home[s] = home[core_blocks[0][s]]
    p.group_home = group_home
    seg_len = np.zeros((slots, n_streams), dtype=np.int64)
    for c in range(n_cores):
        for s in range(slots):
            seg_len[s, :n_subt] = np.maximum(seg_len[s, :n_subt],
                                             blk_q_cnt[core_blocks[c][s]])
    for s in range(slots):
        seg_len[s, n_subt + group_home[s]] = 64
    p.seg_len = seg_len
    seg_start = np.zeros((slots, n_streams), dtype=np.int64)
    cur = np.zeros(n_streams, dtype=np.int64)
    for s in range(slots):
        seg_start[s] = cur
        cur += seg_len[s]
    p.seg_start = seg_start
    stream_len = [int(-(-int(cur[t]) // 128) * 128) for t in range(n_streams)]
    p.stream_len = stream_len
    p.stream_q = [t % n_subt for t in range(n_streams)]
    p.stream_ew = [d if t < n_subt else 2 * d for t in range(n_streams)]

    # gather batch boundaries per stream, in columns. Full gb-col batches,
    # but ramp the tail down so the post-last-gather matmul drain is short.
    p.batch_bounds = []  # [stream] -> list of (col0, ncols)
    for t in range(n_streams):
        cols = stream_len[t] // 128
        bounds = []
        c0 = 0
        if t >= n_subt:
            # self-pair streams: halve into <=2 prefetchable batches
            if cols > 0:
                h = -(-cols // 2)
                bounds.append((0, h))
                if cols - h > 0:
                    bounds.append((h, cols - h))
            p.batch_bounds.append(bounds)
            continue
        while cols - c0 > gb:
            bounds.append((c0, gb))
            c0 += gb
        r = cols - c0
        while r > 6:
            tt = max(6, -(-r // 2))
            bounds.append((c0, tt))
            c0 += tt
            r -= tt
        if r > 0:
            bounds.append((c0, r))
        p.batch_bounds.append(bounds)

    # instance enumeration (program order: slot-major; self pairs first,
    # then per-edge gather columns)
    insts = []  # (slot, stream, col, half)
    slot_inst_range = [0]
    inst_index = {}
    for s in range(slots):
        cnt = 0
        sp = n_subt + int(group_home[s])
        L0 = int(seg_start[s][sp])
        assert seg_len[s][sp] == 64 and L0 % 64 == 0
        col = L0 // 128
        for half in (0, 1):
            inst_index[(s, sp, col, half)] = len(insts)
            insts.append((s, sp, col, half))
            cnt += 2
        for q in range(n_subt):
            L0 = int(seg_start[s][q])
            L1 = L0 + int(seg_len[s][q])
            if L1 > L0:
                for col in range(L0 // 128, -(-L1 // 128)):
                    inst_index[(s, q, col, 0)] = len(insts)
                    insts.append((s, q, col, 0))
                    cnt += 1
        slot_inst_range.append(len(insts))
    p.n_inst = len(insts)
    p.inst_slot = np.asarray([i[0] for i in insts], dtype=np.int64)
    p.inst_stream = np.asarray([i[1] for i in insts], dtype=np.int64)
    p.inst_col = np.asarray([i[2] for i in insts], dtype=np.int64)
    p.inst_half = np.asarray([i[3] for i in insts], dtype=np.int64)
    p.slot_inst_range = slot_inst_range

    # per-core tables
    p.core_idx = []      # [n_cores][stream] int16 wrapped [128, len/16]
    p.core_dst_rel = []  # [n_cores] bf16 [128, n_inst]
    p.core_w = []        # [n_cores] bf16 [128, n_inst]
    for c in range(n_cores):
        idx_t = [np.zeros(stream_len[t], dtype=np.int16)
                 for t in range(n_streams)]
        dst_rel = np.full((128, p.n_inst), -1.0, dtype=np.float32)
        w_lane = np.zeros((128, p.n_inst), dtype=np.float32)
        for s in range(slots):
            b = core_blocks[c][s]
            for q in range(n_subt):
                sl, dl, nm = blk_edges[b][q]
                n_e = len(sl)
                if n_e == 0:
                    continue
                L0 = int(seg_start[s][q])
                idx_t[q][L0:L0 + n_e] = sl
                lanes = L0 + np.arange(n_e)
                cols = lanes // 128
                rows = lanes % 128
                iis = np.asarray(
                    [inst_index[(s, q, int(cc), 0)] for cc in cols],
                    dtype=np.int64)
                dst_rel[rows, iis] = dl
                w_lane[rows, iis] = nm
            # self-loop pairs: 64 lanes, block rows are consecutive in X
            sp = n_subt + int(group_home[s])
            hq = int(group_home[s])
            assert home[b] == hq
            L0 = int(seg_start[s][sp])
            base_pair = (b * 128 - int(p.subt_off[hq])) // 2
            idx_t[sp][L0:L0 + 64] = base_pair + np.arange(64)
            rows = (L0 + np.arange(64)) % 128
            for half in (0, 1):
                ii = inst_index[(s, sp, L0 // 128, half)]
                dst_rel[rows, ii] = 2 * np.arange(64) + half
                w_lane[rows, ii] = p.wself[b * 128 + 2 * np.arange(64) + half]
        idx_wrapped = []
        for t in range(n_streams):
            if stream_len[t] == 0:
                idx_wrapped.append(np.zeros((128, 1), dtype=np.int16))
                continue
            a = idx_t[t].reshape(-1, 16).T  # [16, L/16]
            idx_wrapped.append(np.tile(a, (8, 1)).copy())
        p.core_idx.append(idx_wrapped)
        p.core_dst_rel.append(dst_rel.astype(ml_dtypes.bfloat16))
        p.core_w.append(w_lane.astype(ml_dtypes.bfloat16))

    p.plan_time = time.time() - t0
    return p


# ---------------------------------------------------------------------------
# Device kernel
# ---------------------------------------------------------------------------


def build_nc(p, n_cores=None):
    d = p.d
    gb, sg, ob = p.gb, p.sg, p.ob

    nc = bacc.Bacc("TRN2", target_bir_lowering=False, debug=False,
                   num_devices=n_cores or p.n_cores)

    X = nc.dram_tensor("X", [p.n_pad, d], BF16, kind="ExternalInput")
    WT = nc.dram_tensor("WT", [d, d], BF16, kind="ExternalInput")
    iota = nc.dram_tensor("iota", [128, 128], BF16, kind="ExternalInput")
    dst_rel = nc.dram_tensor("dst_rel", [128, p.n_inst], BF16,
                             kind="ExternalInput")
    w_t = nc.dram_tensor("w_t", [128, p.n_inst], BF16, kind="ExternalInput")
    idx_t = [nc.dram_tensor(f"idx{t}", [128, max(p.stream_len[t] // 16, 1)],
                            I16, kind="ExternalInput")
             for t in range(p.n_streams)]
    out_t = nc.dram_tensor("out", [d, p.slots * 128], BF16,
                           kind="ExternalOutput")

    with tile.TileContext(nc) as tc, ExitStack() as ctx:
        cpool = ctx.enter_context(tc.tile_pool(name="consts", bufs=1))
        WT_sb = cpool.tile([d, d], BF16)
        nc.sync.dma_start(WT_sb[:], WT.ap()[:, :])
        iota_sb = cpool.tile([128, 128], BF16)
        nc.sync.dma_start(iota_sb[:], iota.ap()[:, :])
        dstrel_lo = cpool.tile([128, p.n_inst], BF16)
        nc.sync.dma_start(dstrel_lo[:], dst_rel.ap()[:, :])
        w_lo = cpool.tile([128, p.n_inst], BF16)
        nc.sync.dma_start(w_lo[:], w_t.ap()[:, :])
        dstrel_sb = cpool.tile([128, p.n_inst], F32)
        nc.vector.tensor_scalar_mul(dstrel_sb[:], dstrel_lo[:], 1.0)
        w_sb = cpool.tile([128, p.n_inst], F32)
        nc.vector.tensor_scalar_mul(w_sb[:], w_lo[:], 1.0)
        idx_sb = []
        for t in range(p.n_streams):
            it = cpool.tile([128, idx_t[t].shape[1]], I16, name=f"idxsb{t}")
            nc.sync.dma_start(it[:], idx_t[t].ap()[:, :])
            idx_sb.append(it)

        gpools = [ctx.enter_context(tc.tile_pool(name=f"g{q}", bufs=3))
                  for q in range(p.n_subt)]
        # per-stream pools for the self-pair gathers (all prefetched)
        spools = [ctx.enter_context(tc.tile_pool(name=f"gself{q}", bufs=2))
                  for q in range(p.n_subt)]
        stpool = ctx.enter_context(tc.tile_pool(name="st", bufs=6))
        aggpool = ctx.enter_context(tc.tile_pool(name="agg", bufs=3))
        opool = ctx.enter_context(tc.tile_pool(name="ostage", bufs=4))
        pa = ctx.enter_context(tc.tile_pool(name="pa", bufs=4, space="PSUM"))
        pb = ctx.enter_context(tc.tile_pool(name="pb", bufs=4, space="PSUM"))

        gtiles = {}
        colbatch = []
        for t in range(p.n_streams):
            cb = np.zeros(max(p.stream_len[t] // 128, 1), dtype=np.int64)
            for bi, (c0, ncols) in enumerate(p.batch_bounds[t]):
                cb[c0:c0 + ncols] = bi
            colbatch.append(cb)

        def get_gtile(t, i):
            if (t, i) not in gtiles:
                c0, ncols = p.batch_bounds[t][i]
                lanes = ncols * 128
                ew = p.stream_ew[t]
                q = p.stream_q[t]
                src = X.ap()[int(p.subt_off[q]):int(p.subt_off[q + 1]), :]
                if t >= p.n_subt:
                    src = src.rearrange("(r two) d -> r (two d)", two=2)
                    mx = max(nc2 for _, nc2 in p.batch_bounds[t])
                    gt = spools[q].tile([128, mx * ew], BF16, name="gts")
                else:
                    gt = gpools[q].tile([128, gb * ew], BF16, name=f"gt{q}")
                nc.gpsimd.dma_gather(
                    out_ap=gt[:, 0:ncols * ew].rearrange(
                        "p (j e) -> p j e", e=ew),
                    in_ap=src,
                    idxs_ap=idx_sb[t][:, c0 * 8: c0 * 8 + lanes // 16],
                    num_idxs=lanes,
                    num_idxs_reg=lanes,
                    elem_size=ew,
                    single_packet=False,
                )
                gtiles[(t, i)] = gt
            return gtiles[(t, i)]

        for t in range(p.n_subt, p.n_streams):
            for i in range(len(p.batch_bounds[t])):
                get_gtile(t, i)

        ostage = None
        for s in range(p.slots):
            ps_a = pa.tile([128, d], F32, name="pa_t", space="PSUM")
            i0, i1 = p.slot_inst_range[s], p.slot_inst_range[s + 1]
            for ii in range(i0, i1):
                # weighted one-hot: st[r, j] = (iota[j] == dst_rel[r]) * w[r]
                st_t = stpool.tile([128, 128], BF16, name="st_t")
                nc.vector.tensor_scalar(
                    out=st_t[:], in0=iota_sb[:],
                    scalar1=dstrel_sb[:, ii:ii + 1],
                    scalar2=w_sb[:, ii:ii + 1],
                    op0=mybir.AluOpType.is_equal,
                    op1=mybir.AluOpType.mult)
                t = int(p.inst_stream[ii])
                c = int(p.inst_col[ii])
                bi = int(colbatch[t][c])
                gt = get_gtile(t, bi)
                ew = p.stream_ew[t]
                o = (c - p.batch_bounds[t][bi][0]) * ew \
                    + int(p.inst_half[ii]) * d
                nc.tensor.matmul(out=ps_a[:], lhsT=gt[:, o:o + d],
                                 rhs=st_t[:],
                                 start=(ii == i0), stop=(ii == i1 - 1))
            agg = aggpool.tile([128, d], BF16, name="agg_t")
            if s % 2 == 0:
                nc.scalar.activation(agg[:], ps_a[:],
                                     mybir.ActivationFunctionType.Copy)
            else:
                nc.vector.tensor_scalar_mul(agg[:], ps_a[:], 1.0)
            ps_b = pb.tile([128, d], F32, name="pb_t", space="PSUM")
            nc.tensor.matmul(out=ps_b[:], lhsT=WT_sb[:], rhs=agg[:],
                             start=True, stop=True)
            if s % ob == 0:
                ostage = opool.tile([128, ob * d], BF16, name="ostage")
            ocol = (s % ob) * d
            if s % 2 == 0:
                nc.vector.tensor_scalar_mul(ostage[:, ocol:ocol + d],
                                            ps_b[:], 1.0)
            else:
                nc.scalar.activation(ostage[:, ocol:ocol + d], ps_b[:],
                                     mybir.ActivationFunctionType.Copy)
            if s % ob == ob - 1 or s == p.slots - 1:
                s0 = s // ob * ob
                nsw = s - s0 + 1
                nc.sync.dma_start(
                    out=out_t.ap()[:, s0 * 128:(s0 + nsw) * 128],
                    in_=ostage[:, 0:nsw * d])

    nc.compile()
    return nc


# ---------------------------------------------------------------------------
# Orchestration
# ---------------------------------------------------------------------------


def make_inputs(p, x, W):
    d = p.d
    Xb = np.zeros((p.n_pad, d), dtype=ml_dtypes.bfloat16)
    Xb[:p.n_nodes] = np.asarray(x, dtype=np.float32).astype(ml_dtypes.bfloat16)
    WT = np.ascontiguousarray(np.asarray(W, dtype=np.float32).T).astype(
        ml_dtypes.bfloat16)
    iota = np.broadcast_to(np.arange(128, dtype=np.float32),
                           (128, 128)).astype(ml_dtypes.bfloat16).copy()
    common = {"X": Xb, "WT": WT, "iota": iota}
    in_maps = []
    for c in range(p.n_cores):
        m = dict(common)
        m["dst_rel"] = p.core_dst_rel[c]
        m["w_t"] = p.core_w[c]
        for t in range(p.n_streams):
            m[f"idx{t}"] = p.core_idx[c][t]
        in_maps.append(m)
    return in_maps


def assemble_output(p, results, b):
    out = np.zeros((p.n_nodes, p.d), dtype=np.float32)
    for c in range(p.n_cores):
        oc = np.asarray(results[c]["out"], dtype=np.float32)  # [d, slots*128]
        for s, blk in enumerate(p.core_blocks[c]):
            lo = blk * 128
            if lo >= p.n_nodes:
                continue
            hi = min(lo + 128, p.n_nodes)
            out[lo:hi] = oc[:, s * 128: s * 128 + (hi - lo)].T
    # bias epilogue: out[v] += rdeg[v] * sraw[v] * b
    coef = (p.rdeg[:p.n_nodes] * p.sraw[:p.n_nodes]).astype(np.float32)
    out += coef[:, None] * np.asarray(b, dtype=np.float32)[None, :]
    return out


def gcn_forward(x, edge_index, W, b, n_cores=8, trace=False, **plan_kw):
    n = x.shape[0]
    src = np.asarray(edge_index[0])
    dst = np.asarray(edge_index[1])
    p = build_plan(src, dst, n, n_cores, d=W.shape[0], **plan_kw)
    nc = build_nc(p)
    in_maps = make_inputs(p, x, W)
    res = run_bass_kernel_spmd(nc, in_maps, core_ids=list(range(n_cores)),
                               trace=trace)
    out = assemble_output(p, [r for r in res.results], b)
    return out, p, res


def kernel(x, edge_index, W, b):
    """GCN layer forward on 8 trn2 NeuronCores. Inputs as in setup_inputs()."""
    x = np.asarray(x, dtype=np.float32)
    edge_index = np.asarray(edge_index)
    W = np.asarray(W, dtype=np.float32)
    b = np.asarray(b, dtype=np.float32)
    out, _p, _res = gcn_forward(x, edge_index, W, b, n_cores=N_CORES)
    return out.astype(np.float32)


# revision 33
# speedup vs baseline: 1.9212x; 1.0007x over previous
"""GCN layer (out = D^-1/2 (A+I) D^-1/2 (x W^T + b)) on 8 trn2 NeuronCores.

Strategy (direct-x aggregation, no intermediate table):
  out[dst] = W @ (sum_e norm_e * x[src_e]) + (sum_e norm_e) * b
  with norm_e = rdeg[src]*rdeg[dst].

  - Host: append self-loops, sort edges by dst, partition dst-blocks (128
    nodes) across 8 cores (LPT), build a core-invariant static schedule:
    per (slot, subtable) segments padded at LANE level to the max across
    cores (uniform SPMD program; per-core idx/weight tables).
  - Device: gather raw x rows (bf16, 256B) per edge lane straight from the
    uploaded input table (4 int16-indexable subtable views); accumulate
    aggT[din, dst] = sum_lane x_lane[din] * S[lane, dst] on the PE, where
    S = weighted one-hot (weight = norm_e, built by DVE is_equal * w);
    then one W matmul per dst block; bias term applied on host
    (out += rdeg*sraw*b, with sraw = sum_e rdeg[src]).

  Self-loops are extracted from the edge streams into pair-gathers: each
  block's 128 own nodes are consecutive X rows, fetched as 64 two-node
  512B descriptors (no <512B DMA penalty -> half cost per node) and folded
  in via two extra matmuls per slot.
"""

import math
import time
from contextlib import ExitStack

import ml_dtypes
import numpy as np

import concourse.bass as bass
import concourse.tile as tile
from concourse import bacc, mybir
from concourse.bass_utils import run_bass_kernel_spmd

F32 = mybir.dt.float32
BF16 = mybir.dt.bfloat16
I16 = mybir.dt.int16

N_NODES = 100000
N_EDGES = 1600000
IN_CH = 128
OUT_CH = 128
N_CORES = 8

# ---------------------------------------------------------------------------
# Host-side planning
# ---------------------------------------------------------------------------


class Plan:
    pass


def _optimize_groups(blk_q_cnt, groups, home, n_cores, rounds=60,
                     batch=16384, seed=7):
    """Vectorized local search: swap blocks between slot-groups of the same
    home class to reduce sum over groups of sum_q max (padded lane count)."""
    rng = np.random.default_rng(seed)
    gmat = np.stack([np.asarray(g) for g in groups])          # [G, C]
    G, C = gmat.shape
    ghome = home[gmat[:, 0]]
    obj = blk_q_cnt[gmat].max(axis=1).sum(axis=1)             # [G]
    byhome = {}
    for i in range(G):
        byhome.setdefault(int(ghome[i]), []).append(i)
    classes = [np.asarray(v) for v in byhome.values() if len(v) > 1]
    if not classes:
        return [gmat[i] for i in range(G)]
    for _ in range(rounds):
        # class-matched random group pairs
        cls = [c[rng.integers(0, len(c), size=(batch, 2))] for c in classes]
        pij = np.concatenate(cls, axis=0)
        keep = pij[:, 0] != pij[:, 1]
        pij = pij[keep]
        K = len(pij)
        ab = rng.integers(0, C, size=(K, 2))
        gi = gmat[pij[:, 0]].copy()                           # [K, C]
        gj = gmat[pij[:, 1]].copy()
        bi = gj[np.arange(K), ab[:, 1]].copy()
        gj[np.arange(K), ab[:, 1]] = gi[np.arange(K), ab[:, 0]]
        gi[np.arange(K), ab[:, 0]] = bi
        oi = blk_q_cnt[gi].max(axis=1).sum(axis=1)
        oj = blk_q_cnt[gj].max(axis=1).sum(axis=1)
        delta = (oi + oj) - (obj[pij[:, 0]] + obj[pij[:, 1]])
        good = np.where(delta < 0)[0]
        if len(good) == 0:
            continue
        good = good[np.argsort(delta[good])]
        used = np.zeros(G, dtype=bool)
        for k in good:
            i, j = pij[k]
            if used[i] or used[j]:
                continue
            used[i] = used[j] = True
            gmat[i] = gi[k]
            gmat[j] = gj[k]
            obj[i] = oi[k]
            obj[j] = oj[k]
    return [gmat[i] for i in range(G)]


def build_plan(src, dst, n_nodes, n_cores, d=128, gb=16, sg=8, ob=8,
               subt_cap=32768):
    """src/dst: REAL edge endpoints (self loops handled separately)."""
    t0 = time.time()
    p = Plan()
    p.d = d
    p.gb = gb
    p.sg = sg
    p.ob = ob
    p.n_nodes = n_nodes
    p.n_cores = n_cores

    n_blocks = math.ceil(n_nodes / 128)
    slots = math.ceil(n_blocks / n_cores)
    n_blocks = slots * n_cores
    p.slots = slots
    p.n_blocks = n_blocks
    p.n_pad = n_blocks * 128

    # subtables over node id range (int16 gather index limit)
    n_subt = math.ceil(p.n_pad / subt_cap)
    off = [min(q * subt_cap, p.n_pad) for q in range(n_subt + 1)]
    p.subt_off = np.asarray(off)
    p.n_subt = n_subt

    src = np.asarray(src, dtype=np.int64)
    dst = np.asarray(dst, dtype=np.int64)
    loop = np.arange(n_nodes, dtype=np.int64)

    # degrees / norms (host; exactly mirrors the reference formula,
    # self-loops included in the degree)
    deg = np.bincount(np.concatenate([dst, loop]),
                      minlength=p.n_pad).astype(np.float64)
    deg[n_nodes:] = 1.0
    deg[deg == 0] = 1.0
    rdeg = 1.0 / np.sqrt(deg)
    p.rdeg = rdeg.astype(np.float32)
    norm_real = (rdeg[src] * rdeg[dst]).astype(np.float32)
    # host bias epilogue: out[v] += rdeg[v] * sraw[v] * b  (incl. self loop)
    sraw = np.zeros(p.n_pad, dtype=np.float64)
    np.add.at(sraw, dst, rdeg[src])
    sraw[:n_nodes] += rdeg[:n_nodes]
    p.sraw = sraw.astype(np.float32)
    # self-loop weight per node (x[v] contribution), 0 for pad nodes
    wself = (rdeg * rdeg).astype(np.float32)
    wself[n_nodes:] = 0.0
    p.wself = wself

    # sort REAL edges by dst
    order = np.argsort(dst, kind="stable")
    dst_s = dst[order]
    src_s = src[order]
    norm_s = norm_real[order]
    blk_ptr = np.searchsorted(dst_s, np.arange(0, p.n_pad + 1, 128))
    blk_cnt = blk_ptr[1:] - blk_ptr[:-1]

    # per (block, q) edge sublists
    subt_of_src = np.searchsorted(p.subt_off[1:], src_s, side="right")
    blk_edges = []  # [block][q] -> (src_local, dst_local, norm)
    blk_q_cnt = np.zeros((n_blocks, n_subt), dtype=np.int64)
    for b in range(n_blocks):
        lo, hi = blk_ptr[b], blk_ptr[b + 1]
        qs = subt_of_src[lo:hi]
        per_q = []
        for q in range(n_subt):
            m = qs == q
            sl = (src_s[lo:hi][m] - p.subt_off[q]).astype(np.int16)
            dl = (dst_s[lo:hi][m] - b * 128).astype(np.int32)
            nm = norm_s[lo:hi][m]
            per_q.append((sl, dl, nm))
            blk_q_cnt[b, q] = len(sl)
        blk_edges.append(per_q)

    # Slot grouping: blocks sharing a slot should have near-identical per-q
    # edge counts, since seg_len[s][q] = max over the group. Self-loops
    # concentrate ~128 edges in a block's home subtable, so group by
    # (home subtable, size rank); within a slot, biggest block goes to the
    # least-loaded core (balances per-core totals).
    home = np.searchsorted(p.subt_off[1:], np.arange(n_blocks) * 128,
                           side="right")
    groups = []
    for q in range(n_subt):
        hb = np.where(home == q)[0]
        hb = hb[np.argsort(-blk_cnt[hb], kind="stable")]
        assert len(hb) % n_cores == 0, (q, len(hb))
        for i in range(0, len(hb), n_cores):
            groups.append(hb[i:i + n_cores])
    assert len(groups) == slots
    groups = _optimize_groups(blk_q_cnt, groups, home, n_cores)
    core_loads = np.zeros(n_cores, dtype=np.int64)
    core_blocks = [[-1] * slots for _ in range(n_cores)]
    for s, group in enumerate(groups):
        free = list(range(n_cores))
        group = sorted(group, key=lambda b: -int(blk_cnt[b]))
        for b in group:  # desc by size
            c = min(free, key=lambda c: core_loads[c])
            free.remove(c)
            core_loads[c] += int(blk_cnt[b])
            core_blocks[c][s] = int(b)
    p.core_blocks = core_blocks

    # Streams 0..n_subt-1: per-edge gathers (elem = d, one idx per edge).
    # Streams n_subt..2*n_subt-1: self-loop pair gathers (elem = 2d, one idx
    # per CONSECUTIVE NODE PAIR of the slot's own block; 64 lanes per slot,
    # in the block's home subtable). 512B descriptors avoid the <512B DMA
    # penalty, halving the per-node self-loop gather cost.
    n_streams = 2 * n_subt
    p.n_streams = n_streams
    group_home = np.zeros(slots, dtype=np.int64)
    for s in range(slots):
        group_home[s] = home[core_blocks[0][s]]
    p.group_home = group_home
    seg_len = np.zeros((slots, n_streams), dtype=np.int64)
    for c in range(n_cores):
        for s in range(slots):
            seg_len[s, :n_subt] = np.maximum(seg_len[s, :n_subt],
                                             blk_q_cnt[core_blocks[c][s]])
    for s in range(slots):
        seg_len[s, n_subt + group_home[s]] = 64
    p.seg_len = seg_len
    seg_start = np.zeros((slots, n_streams), dtype=np.int64)
    cur = np.zeros(n_streams, dtype=np.int64)
    for s in range(slots):
        seg_start[s] = cur
        cur += seg_len[s]
    p.seg_start = seg_start
    stream_len = [int(-(-int(cur[t]) // 128) * 128) for t in range(n_streams)]
    p.stream_len = stream_len
    p.stream_q = [t % n_subt for t in range(n_streams)]
    p.stream_ew = [d if t < n_subt else 2 * d for t in range(n_streams)]

    # gather batch boundaries per stream, in columns. Full gb-col batches,
    # but ramp the tail down so the post-last-gather matmul drain is short.
    p.batch_bounds = []  # [stream] -> list of (col0, ncols)
    for t in range(n_streams):
        cols = stream_len[t] // 128
        bounds = []
        c0 = 0
        if t >= n_subt:
            # self-pair streams: halve into <=2 prefetchable batches
            if cols > 0:
                h = -(-cols // 2)
                bounds.append((0, h))
                if cols - h > 0:
                    bounds.append((h, cols - h))
            p.batch_bounds.append(bounds)
            continue
        while cols - c0 > gb:
            bounds.append((c0, gb))
            c0 += gb
        r = cols - c0
        while r > 6:
            tt = max(6, -(-r // 2))
            bounds.append((c0, tt))
            c0 += tt
            r -= tt
        if r > 0:
            bounds.append((c0, r))
        p.batch_bounds.append(bounds)

    # instance enumeration (program order: slot-major; self pairs first,
    # then per-edge gather columns)
    insts = []  # (slot, stream, col, half)
    slot_inst_range = [0]
    inst_index = {}
    for s in range(slots):
        cnt = 0
        sp = n_subt + int(group_home[s])
        L0 = int(seg_start[s][sp])
        assert seg_len[s][sp] == 64 and L0 % 64 == 0
        col = L0 // 128
        for half in (0, 1):
            inst_index[(s, sp, col, half)] = len(insts)
            insts.append((s, sp, col, half))
            cnt += 2
        for q in range(n_subt):
            L0 = int(seg_start[s][q])
            L1 = L0 + int(seg_len[s][q])
            if L1 > L0:
                for col in range(L0 // 128, -(-L1 // 128)):
                    inst_index[(s, q, col, 0)] = len(insts)
                    insts.append((s, q, col, 0))
                    cnt += 1
        slot_inst_range.append(len(insts))
    p.n_inst = len(insts)
    p.inst_slot = np.asarray([i[0] for i in insts], dtype=np.int64)
    p.inst_stream = np.asarray([i[1] for i in insts], dtype=np.int64)
    p.inst_col = np.asarray([i[2] for i in insts], dtype=np.int64)
    p.inst_half = np.asarray([i[3] for i in insts], dtype=np.int64)
    p.slot_inst_range = slot_inst_range

    # per-core tables
    p.core_idx = []      # [n_cores][stream] int16 wrapped [128, len/16]
    p.core_dst_rel = []  # [n_cores] bf16 [128, n_inst]
    p.core_w = []        # [n_cores] bf16 [128, n_inst]
    for c in range(n_cores):
        idx_t = [np.zeros(stream_len[t], dtype=np.int16)
                 for t in range(n_streams)]
        dst_rel = np.full((128, p.n_inst), -1.0, dtype=np.float32)
        w_lane = np.zeros((128, p.n_inst), dtype=np.float32)
        for s in range(slots):
            b = core_blocks[c][s]
            for q in range(n_subt):
                sl, dl, nm = blk_edges[b][q]
                n_e = len(sl)
                if n_e == 0:
                    continue
                L0 = int(seg_start[s][q])
                idx_t[q][L0:L0 + n_e] = sl
                lanes = L0 + np.arange(n_e)
                cols = lanes // 128
                rows = lanes % 128
                iis = np.asarray(
                    [inst_index[(s, q, int(cc), 0)] for cc in cols],
                    dtype=np.int64)
                dst_rel[rows, iis] = dl
                w_lane[rows, iis] = nm
            # self-loop pairs: 64 lanes, block rows are consecutive in X
            sp = n_subt + int(group_home[s])
            hq = int(group_home[s])
            assert home[b] == hq
            L0 = int(seg_start[s][sp])
            base_pair = (b * 128 - int(p.subt_off[hq])) // 2
            idx_t[sp][L0:L0 + 64] = base_pair + np.arange(64)
            rows = (L0 + np.arange(64)) % 128
            for half in (0, 1):
                ii = inst_index[(s, sp, L0 // 128, half)]
                dst_rel[rows, ii] = 2 * np.arange(64) + half
                w_lane[rows, ii] = p.wself[b * 128 + 2 * np.arange(64) + half]
        idx_wrapped = []
        for t in range(n_streams):
            if stream_len[t] == 0:
                idx_wrapped.append(np.zeros((128, 1), dtype=np.int16))
                continue
            a = idx_t[t].reshape(-1, 16).T  # [16, L/16]
            idx_wrapped.append(np.tile(a, (8, 1)).copy())
        p.core_idx.append(idx_wrapped)
        p.core_dst_rel.append(dst_rel.astype(ml_dtypes.bfloat16))
        p.core_w.append(w_lane.astype(ml_dtypes.bfloat16))

    p.plan_time = time.time() - t0
    return p


# ---------------------------------------------------------------------------
# Device kernel
# ---------------------------------------------------------------------------


def build_nc(p, n_cores=None):
    d = p.d
    gb, sg, ob = p.gb, p.sg, p.ob

    nc = bacc.Bacc("TRN2", target_bir_lowering=False, debug=False,
                   num_devices=n_cores or p.n_cores)

    X = nc.dram_tensor("X", [p.n_pad, d], BF16, kind="ExternalInput")
    WT = nc.dram_tensor("WT", [d, d], BF16, kind="ExternalInput")
    iota = nc.dram_tensor("iota", [128, 128], BF16, kind="ExternalInput")
    dst_rel = nc.dram_tensor("dst_rel", [128, p.n_inst], BF16,
                             kind="ExternalInput")
    w_t = nc.dram_tensor("w_t", [128, p.n_inst], BF16, kind="ExternalInput")
    idx_t = [nc.dram_tensor(f"idx{t}", [128, max(p.stream_len[t] // 16, 1)],
                            I16, kind="ExternalInput")
             for t in range(p.n_streams)]
    out_t = nc.dram_tensor("out", [d, p.slots * 128], BF16,
                           kind="ExternalOutput")

    with tile.TileContext(nc) as tc, ExitStack() as ctx:
        cpool = ctx.enter_context(tc.tile_pool(name="consts", bufs=1))
        WT_sb = cpool.tile([d, d], BF16)
        nc.sync.dma_start(WT_sb[:], WT.ap()[:, :])
        iota_sb = cpool.tile([128, 128], BF16)
        nc.sync.dma_start(iota_sb[:], iota.ap()[:, :])
        dstrel_lo = cpool.tile([128, p.n_inst], BF16)
        nc.sync.dma_start(dstrel_lo[:], dst_rel.ap()[:, :])
        w_lo = cpool.tile([128, p.n_inst], BF16)
        nc.sync.dma_start(w_lo[:], w_t.ap()[:, :])
        dstrel_sb = cpool.tile([128, p.n_inst], F32)
        nc.vector.tensor_scalar_mul(dstrel_sb[:], dstrel_lo[:], 1.0)
        w_sb = cpool.tile([128, p.n_inst], F32)
        nc.vector.tensor_scalar_mul(w_sb[:], w_lo[:], 1.0)
        idx_sb = []
        for t in range(p.n_streams):
            it = cpool.tile([128, idx_t[t].shape[1]], I16, name=f"idxsb{t}")
            nc.sync.dma_start(it[:], idx_t[t].ap()[:, :])
            idx_sb.append(it)

        gpools = [ctx.enter_context(tc.tile_pool(name=f"g{q}", bufs=4))
                  for q in range(p.n_subt)]
        # per-stream pools for the self-pair gathers (all prefetched)
        spools = [ctx.enter_context(tc.tile_pool(name=f"gself{q}", bufs=2))
                  for q in range(p.n_subt)]
        stpool = ctx.enter_context(tc.tile_pool(name="st", bufs=6))
        aggpool = ctx.enter_context(tc.tile_pool(name="agg", bufs=3))
        opool = ctx.enter_context(tc.tile_pool(name="ostage", bufs=4))
        pa = ctx.enter_context(tc.tile_pool(name="pa", bufs=4, space="PSUM"))
        pb = ctx.enter_context(tc.tile_pool(name="pb", bufs=4, space="PSUM"))

        gtiles = {}
        colbatch = []
        for t in range(p.n_streams):
            cb = np.zeros(max(p.stream_len[t] // 128, 1), dtype=np.int64)
            for bi, (c0, ncols) in enumerate(p.batch_bounds[t]):
                cb[c0:c0 + ncols] = bi
            colbatch.append(cb)

        def get_gtile(t, i):
            if (t, i) not in gtiles:
                c0, ncols = p.batch_bounds[t][i]
                lanes = ncols * 128
                ew = p.stream_ew[t]
                q = p.stream_q[t]
                src = X.ap()[int(p.subt_off[q]):int(p.subt_off[q + 1]), :]
                if t >= p.n_subt:
                    src = src.rearrange("(r two) d -> r (two d)", two=2)
                    mx = max(nc2 for _, nc2 in p.batch_bounds[t])
                    gt = spools[q].tile([128, mx * ew], BF16, name="gts")
                else:
                    gt = gpools[q].tile([128, gb * ew], BF16, name=f"gt{q}")
                nc.gpsimd.dma_gather(
                    out_ap=gt[:, 0:ncols * ew].rearrange(
                        "p (j e) -> p j e", e=ew),
                    in_ap=src,
                    idxs_ap=idx_sb[t][:, c0 * 8: c0 * 8 + lanes // 16],
                    num_idxs=lanes,
                    num_idxs_reg=lanes,
                    elem_size=ew,
                    single_packet=False,
                )
                gtiles[(t, i)] = gt
            return gtiles[(t, i)]

        for t in range(p.n_subt, p.n_streams):
            for i in range(len(p.batch_bounds[t])):
                get_gtile(t, i)

        ostage = None
        for s in range(p.slots):
            ps_a = pa.tile([128, d], F32, name="pa_t", space="PSUM")
            i0, i1 = p.slot_inst_range[s], p.slot_inst_range[s + 1]
            for ii in range(i0, i1):
                # weighted one-hot: st[r, j] = (iota[j] == dst_rel[r]) * w[r]
                st_t = stpool.tile([128, 128], BF16, name="st_t")
                nc.vector.tensor_scalar(
                    out=st_t[:], in0=iota_sb[:],
                    scalar1=dstrel_sb[:, ii:ii + 1],
                    scalar2=w_sb[:, ii:ii + 1],
                    op0=mybir.AluOpType.is_equal,
                    op1=mybir.AluOpType.mult)
                t = int(p.inst_stream[ii])
                c = int(p.inst_col[ii])
                bi = int(colbatch[t][c])
                gt = get_gtile(t, bi)
                ew = p.stream_ew[t]
                o = (c - p.batch_bounds[t][bi][0]) * ew \
                    + int(p.inst_half[ii]) * d
                nc.tensor.matmul(out=ps_a[:], lhsT=gt[:, o:o + d],
                                 rhs=st_t[:],
                                 start=(ii == i0), stop=(ii == i1 - 1))
            agg = aggpool.tile([128, d], BF16, name="agg_t")
            if s % 2 == 0:
                nc.scalar.activation(agg[:], ps_a[:],
                                     mybir.ActivationFunctionType.Copy)
            else:
                nc.vector.tensor_scalar_mul(agg[:], ps_a[:], 1.0)
            ps_b = pb.tile([128, d], F32, name="pb_t", space="PSUM")
            nc.tensor.matmul(out=ps_b[:], lhsT=WT_sb[:], rhs=agg[:],
                             start=True, stop=True)
            if s % ob == 0:
                ostage = opool.tile([128, ob * d], BF16, name="ostage")
            ocol = (s % ob) * d
            if s % 2 == 0:
                nc.vector.tensor_scalar_mul(ostage[:, ocol:ocol + d],
                                            ps_b[:], 1.0)
            else:
                nc.scalar.activation(ostage[:, ocol:ocol + d], ps_b[:],
                                     mybir.ActivationFunctionType.Copy)
            if s % ob == ob - 1 or s == p.slots - 1:
                s0 = s // ob * ob
                nsw = s - s0 + 1
                nc.sync.dma_start(
                    out=out_t.ap()[:, s0 * 128:(s0 + nsw) * 128],
                    in_=ostage[:, 0:nsw * d])

    nc.compile()
    return nc


# ---------------------------------------------------------------------------
# Orchestration
# ---------------------------------------------------------------------------


def make_inputs(p, x, W):
    d = p.d
    Xb = np.zeros((p.n_pad, d), dtype=ml_dtypes.bfloat16)
    Xb[:p.n_nodes] = np.asarray(x, dtype=np.float32).astype(ml_dtypes.bfloat16)
    WT = np.ascontiguousarray(np.asarray(W, dtype=np.float32).T).astype(
        ml_dtypes.bfloat16)
    iota = np.broadcast_to(np.arange(128, dtype=np.float32),
                           (128, 128)).astype(ml_dtypes.bfloat16).copy()
    common = {"X": Xb, "WT": WT, "iota": iota}
    in_maps = []
    for c in range(p.n_cores):
        m = dict(common)
        m["dst_rel"] = p.core_dst_rel[c]
        m["w_t"] = p.core_w[c]
        for t in range(p.n_streams):
            m[f"idx{t}"] = p.core_idx[c][t]
        in_maps.append(m)
    return in_maps


def assemble_output(p, results, b):
    out = np.zeros((p.n_nodes, p.d), dtype=np.float32)
    for c in range(p.n_cores):
        oc = np.asarray(results[c]["out"], dtype=np.float32)  # [d, slots*128]
        for s, blk in enumerate(p.core_blocks[c]):
            lo = blk * 128
            if lo >= p.n_nodes:
                continue
            hi = min(lo + 128, p.n_nodes)
            out[lo:hi] = oc[:, s * 128: s * 128 + (hi - lo)].T
    # bias epilogue: out[v] += rdeg[v] * sraw[v] * b
    coef = (p.rdeg[:p.n_nodes] * p.sraw[:p.n_nodes]).astype(np.float32)
    out += coef[:, None] * np.asarray(b, dtype=np.float32)[None, :]
    return out


def gcn_forward(x, edge_index, W, b, n_cores=8, trace=False, **plan_kw):
    n = x.shape[0]
    src = np.asarray(edge_index[0])
    dst = np.asarray(edge_index[1])
    p = build_plan(src, dst, n, n_cores, d=W.shape[0], **plan_kw)
    nc = build_nc(p)
    in_maps = make_inputs(p, x, W)
    res = run_bass_kernel_spmd(nc, in_maps, core_ids=list(range(n_cores)),
                               trace=trace)
    out = assemble_output(p, [r for r in res.results], b)
    return out, p, res


def kernel(x, edge_index, W, b):
    """GCN layer forward on 8 trn2 NeuronCores. Inputs as in setup_inputs()."""
    x = np.asarray(x, dtype=np.float32)
    edge_index = np.asarray(edge_index)
    W = np.asarray(W, dtype=np.float32)
    b = np.asarray(b, dtype=np.float32)
    out, _p, _res = gcn_forward(x, edge_index, W, b, n_cores=N_CORES)
    return out.astype(np.float32)


# revision 34
# speedup vs baseline: 1.9824x; 1.0318x over previous
"""GCN layer (out = D^-1/2 (A+I) D^-1/2 (x W^T + b)) on 8 trn2 NeuronCores.

Strategy (direct-x aggregation, no intermediate table):
  out[dst] = W @ (sum_e norm_e * x[src_e]) + (sum_e norm_e) * b
  with norm_e = rdeg[src]*rdeg[dst].

  - Host: append self-loops, sort edges by dst, partition dst-blocks (128
    nodes) across 8 cores (LPT), build a core-invariant static schedule:
    per (slot, subtable) segments padded at LANE level to the max across
    cores (uniform SPMD program; per-core idx/weight tables).
  - Device: gather raw x rows (bf16, 256B) per edge lane straight from the
    uploaded input table (4 int16-indexable subtable views); accumulate
    aggT[din, dst] = sum_lane x_lane[din] * S[lane, dst] on the PE, where
    S = weighted one-hot (weight = norm_e, built by DVE is_equal * w);
    then one W matmul per dst block; bias term applied on host
    (out += rdeg*sraw*b, with sraw = sum_e rdeg[src]).

  Self-loops are extracted from the edge streams into pair-gathers: each
  block's 128 own nodes are consecutive X rows, fetched as 64 two-node
  512B descriptors (no <512B DMA penalty -> half cost per node) and folded
  in via two extra matmuls per slot.
"""

import math
import time
from contextlib import ExitStack

import ml_dtypes
import numpy as np

import concourse.bass as bass
import concourse.tile as tile
from concourse import bacc, mybir
from concourse.bass_utils import run_bass_kernel_spmd

F32 = mybir.dt.float32
BF16 = mybir.dt.bfloat16
I16 = mybir.dt.int16

N_NODES = 100000
N_EDGES = 1600000
IN_CH = 128
OUT_CH = 128
N_CORES = 8

# ---------------------------------------------------------------------------
# Host-side planning
# ---------------------------------------------------------------------------


class Plan:
    pass


def _optimize_groups(blk_q_cnt, groups, home, n_cores, rounds=60,
                     batch=16384, seed=7):
    """Vectorized local search: swap blocks between slot-groups of the same
    home class to reduce sum over groups of sum_q max (padded lane count)."""
    rng = np.random.default_rng(seed)
    gmat = np.stack([np.asarray(g) for g in groups])          # [G, C]
    G, C = gmat.shape
    ghome = home[gmat[:, 0]]
    obj = blk_q_cnt[gmat].max(axis=1).sum(axis=1)             # [G]
    byhome = {}
    for i in range(G):
        byhome.setdefault(int(ghome[i]), []).append(i)
    classes = [np.asarray(v) for v in byhome.values() if len(v) > 1]
    if not classes:
        return [gmat[i] for i in range(G)]
    for _ in range(rounds):
        # class-matched random group pairs
        cls = [c[rng.integers(0, len(c), size=(batch, 2))] for c in classes]
        pij = np.concatenate(cls, axis=0)
        keep = pij[:, 0] != pij[:, 1]
        pij = pij[keep]
        K = len(pij)
        ab = rng.integers(0, C, size=(K, 2))
        gi = gmat[pij[:, 0]].copy()                           # [K, C]
        gj = gmat[pij[:, 1]].copy()
        bi = gj[np.arange(K), ab[:, 1]].copy()
        gj[np.arange(K), ab[:, 1]] = gi[np.arange(K), ab[:, 0]]
        gi[np.arange(K), ab[:, 0]] = bi
        oi = blk_q_cnt[gi].max(axis=1).sum(axis=1)
        oj = blk_q_cnt[gj].max(axis=1).sum(axis=1)
        delta = (oi + oj) - (obj[pij[:, 0]] + obj[pij[:, 1]])
        good = np.where(delta < 0)[0]
        if len(good) == 0:
            continue
        good = good[np.argsort(delta[good])]
        used = np.zeros(G, dtype=bool)
        for k in good:
            i, j = pij[k]
            if used[i] or used[j]:
                continue
            used[i] = used[j] = True
            gmat[i] = gi[k]
            gmat[j] = gj[k]
            obj[i] = oi[k]
            obj[j] = oj[k]
    return [gmat[i] for i in range(G)]


def build_plan(src, dst, n_nodes, n_cores, d=128, gb=16, sg=8, ob=8,
               subt_cap=32768):
    """src/dst: REAL edge endpoints (self loops handled separately)."""
    t0 = time.time()
    p = Plan()
    p.d = d
    p.gb = gb
    p.sg = sg
    p.ob = ob
    p.n_nodes = n_nodes
    p.n_cores = n_cores

    n_blocks = math.ceil(n_nodes / 128)
    slots = math.ceil(n_blocks / n_cores)
    n_blocks = slots * n_cores
    p.slots = slots
    p.n_blocks = n_blocks
    p.n_pad = n_blocks * 128

    # subtables over node id range (int16 gather index limit)
    n_subt = math.ceil(p.n_pad / subt_cap)
    off = [min(q * subt_cap, p.n_pad) for q in range(n_subt + 1)]
    p.subt_off = np.asarray(off)
    p.n_subt = n_subt

    src = np.asarray(src, dtype=np.int64)
    dst = np.asarray(dst, dtype=np.int64)
    loop = np.arange(n_nodes, dtype=np.int64)

    # degrees / norms (host; exactly mirrors the reference formula,
    # self-loops included in the degree)
    deg = np.bincount(np.concatenate([dst, loop]),
                      minlength=p.n_pad).astype(np.float64)
    deg[n_nodes:] = 1.0
    deg[deg == 0] = 1.0
    rdeg = 1.0 / np.sqrt(deg)
    p.rdeg = rdeg.astype(np.float32)
    norm_real = (rdeg[src] * rdeg[dst]).astype(np.float32)
    # host bias epilogue: out[v] += rdeg[v] * sraw[v] * b  (incl. self loop)
    sraw = np.zeros(p.n_pad, dtype=np.float64)
    np.add.at(sraw, dst, rdeg[src])
    sraw[:n_nodes] += rdeg[:n_nodes]
    p.sraw = sraw.astype(np.float32)
    # self-loop weight per node (x[v] contribution), 0 for pad nodes
    wself = (rdeg * rdeg).astype(np.float32)
    wself[n_nodes:] = 0.0
    p.wself = wself

    # sort REAL edges by dst
    order = np.argsort(dst, kind="stable")
    dst_s = dst[order]
    src_s = src[order]
    norm_s = norm_real[order]
    blk_ptr = np.searchsorted(dst_s, np.arange(0, p.n_pad + 1, 128))
    blk_cnt = blk_ptr[1:] - blk_ptr[:-1]

    # per (block, q) edge sublists
    subt_of_src = np.searchsorted(p.subt_off[1:], src_s, side="right")
    blk_edges = []  # [block][q] -> (src_local, dst_local, norm)
    blk_q_cnt = np.zeros((n_blocks, n_subt), dtype=np.int64)
    for b in range(n_blocks):
        lo, hi = blk_ptr[b], blk_ptr[b + 1]
        qs = subt_of_src[lo:hi]
        per_q = []
        for q in range(n_subt):
            m = qs == q
            sl = (src_s[lo:hi][m] - p.subt_off[q]).astype(np.int16)
            dl = (dst_s[lo:hi][m] - b * 128).astype(np.int32)
            nm = norm_s[lo:hi][m]
            per_q.append((sl, dl, nm))
            blk_q_cnt[b, q] = len(sl)
        blk_edges.append(per_q)

    # Slot grouping: blocks sharing a slot should have near-identical per-q
    # edge counts, since seg_len[s][q] = max over the group. Self-loops
    # concentrate ~128 edges in a block's home subtable, so group by
    # (home subtable, size rank); within a slot, biggest block goes to the
    # least-loaded core (balances per-core totals).
    home = np.searchsorted(p.subt_off[1:], np.arange(n_blocks) * 128,
                           side="right")
    groups = []
    for q in range(n_subt):
        hb = np.where(home == q)[0]
        hb = hb[np.argsort(-blk_cnt[hb], kind="stable")]
        assert len(hb) % n_cores == 0, (q, len(hb))
        for i in range(0, len(hb), n_cores):
            groups.append(hb[i:i + n_cores])
    assert len(groups) == slots
    groups = _optimize_groups(blk_q_cnt, groups, home, n_cores)
    core_loads = np.zeros(n_cores, dtype=np.int64)
    core_blocks = [[-1] * slots for _ in range(n_cores)]
    for s, group in enumerate(groups):
        free = list(range(n_cores))
        group = sorted(group, key=lambda b: -int(blk_cnt[b]))
        for b in group:  # desc by size
            c = min(free, key=lambda c: core_loads[c])
            free.remove(c)
            core_loads[c] += int(blk_cnt[b])
            core_blocks[c][s] = int(b)
    p.core_blocks = core_blocks

    # Streams 0..n_subt-1: per-edge gathers (elem = d, one idx per edge).
    # Streams n_subt..2*n_subt-1: self-loop pair gathers (elem = 2d, one idx
    # per CONSECUTIVE NODE PAIR of the slot's own block; 64 lanes per slot,
    # in the block's home subtable). 512B descriptors avoid the <512B DMA
    # penalty, halving the per-node self-loop gather cost.
    n_streams = 2 * n_subt
    p.n_streams = n_streams
    group_home = np.zeros(slots, dtype=np.int64)
    for s in range(slots):
        group_home[s] = home[core_blocks[0][s]]
    p.group_home = group_home
    seg_len = np.zeros((slots, n_streams), dtype=np.int64)
    for c in range(n_cores):
        for s in range(slots):
            seg_len[s, :n_subt] = np.maximum(seg_len[s, :n_subt],
                                             blk_q_cnt[core_blocks[c][s]])
    for s in range(slots):
        seg_len[s, n_subt + group_home[s]] = 64
    p.seg_len = seg_len
    seg_start = np.zeros((slots, n_streams), dtype=np.int64)
    cur = np.zeros(n_streams, dtype=np.int64)
    for s in range(slots):
        seg_start[s] = cur
        cur += seg_len[s]
    p.seg_start = seg_start
    stream_len = [int(-(-int(cur[t]) // 128) * 128) for t in range(n_streams)]
    p.stream_len = stream_len
    p.stream_q = [t % n_subt for t in range(n_streams)]
    p.stream_ew = [d if t < n_subt else 2 * d for t in range(n_streams)]

    # gather batch boundaries per stream, in columns. Full gb-col batches,
    # but ramp the tail down so the post-last-gather matmul drain is short.
    p.batch_bounds = []  # [stream] -> list of (col0, ncols)
    for t in range(n_streams):
        cols = stream_len[t] // 128
        bounds = []
        c0 = 0
        if t >= n_subt:
            # self-pair streams: halve into <=2 prefetchable batches
            if cols > 0:
                h = -(-cols // 2)
                bounds.append((0, h))
                if cols - h > 0:
                    bounds.append((h, cols - h))
            p.batch_bounds.append(bounds)
            continue
        while cols - c0 > gb:
            bounds.append((c0, gb))
            c0 += gb
        r = cols - c0
        while r > 6:
            tt = max(6, -(-r // 2))
            bounds.append((c0, tt))
            c0 += tt
            r -= tt
        if r > 0:
            bounds.append((c0, r))
        p.batch_bounds.append(bounds)

    # instance enumeration (program order: slot-major; self pairs first,
    # then per-edge gather columns)
    insts = []  # (slot, stream, col, half)
    slot_inst_range = [0]
    inst_index = {}
    for s in range(slots):
        cnt = 0
        sp = n_subt + int(group_home[s])
        L0 = int(seg_start[s][sp])
        assert seg_len[s][sp] == 64 and L0 % 64 == 0
        col = L0 // 128
        for half in (0, 1):
            inst_index[(s, sp, col, half)] = len(insts)
            insts.append((s, sp, col, half))
            cnt += 2
        for q in range(n_subt):
            L0 = int(seg_start[s][q])
            L1 = L0 + int(seg_len[s][q])
            if L1 > L0:
                for col in range(L0 // 128, -(-L1 // 128)):
                    inst_index[(s, q, col, 0)] = len(insts)
                    insts.append((s, q, col, 0))
                    cnt += 1
        slot_inst_range.append(len(insts))
    p.n_inst = len(insts)
    p.inst_slot = np.asarray([i[0] for i in insts], dtype=np.int64)
    p.inst_stream = np.asarray([i[1] for i in insts], dtype=np.int64)
    p.inst_col = np.asarray([i[2] for i in insts], dtype=np.int64)
    p.inst_half = np.asarray([i[3] for i in insts], dtype=np.int64)
    p.slot_inst_range = slot_inst_range

    # per-core tables
    p.core_idx = []      # [n_cores][stream] int16 wrapped [128, len/16]
    p.core_dst_rel = []  # [n_cores] bf16 [128, n_inst]
    p.core_w = []        # [n_cores] bf16 [128, n_inst]
    for c in range(n_cores):
        idx_t = [np.zeros(stream_len[t], dtype=np.int16)
                 for t in range(n_streams)]
        dst_rel = np.full((128, p.n_inst), -1.0, dtype=np.float32)
        w_lane = np.zeros((128, p.n_inst), dtype=np.float32)
        for s in range(slots):
            b = core_blocks[c][s]
            for q in range(n_subt):
                sl, dl, nm = blk_edges[b][q]
                n_e = len(sl)
                if n_e == 0:
                    continue
                L0 = int(seg_start[s][q])
                idx_t[q][L0:L0 + n_e] = sl
                lanes = L0 + np.arange(n_e)
                cols = lanes // 128
                rows = lanes % 128
                iis = np.asarray(
                    [inst_index[(s, q, int(cc), 0)] for cc in cols],
                    dtype=np.int64)
                dst_rel[rows, iis] = dl
                w_lane[rows, iis] = nm
            # self-loop pairs: 64 lanes, block rows are consecutive in X
            sp = n_subt + int(group_home[s])
            hq = int(group_home[s])
            assert home[b] == hq
            L0 = int(seg_start[s][sp])
            base_pair = (b * 128 - int(p.subt_off[hq])) // 2
            idx_t[sp][L0:L0 + 64] = base_pair + np.arange(64)
            rows = (L0 + np.arange(64)) % 128
            for half in (0, 1):
                ii = inst_index[(s, sp, L0 // 128, half)]
                dst_rel[rows, ii] = 2 * np.arange(64) + half
                w_lane[rows, ii] = p.wself[b * 128 + 2 * np.arange(64) + half]
        idx_wrapped = []
        for t in range(n_streams):
            if stream_len[t] == 0:
                idx_wrapped.append(np.zeros((128, 1), dtype=np.int16))
                continue
            a = idx_t[t].reshape(-1, 16).T  # [16, L/16]
            idx_wrapped.append(np.tile(a, (8, 1)).copy())
        p.core_idx.append(idx_wrapped)
        p.core_dst_rel.append(dst_rel.astype(ml_dtypes.bfloat16))
        p.core_w.append(w_lane.astype(ml_dtypes.bfloat16))

    p.plan_time = time.time() - t0
    return p


# ---------------------------------------------------------------------------
# Device kernel
# ---------------------------------------------------------------------------


def build_nc(p, n_cores=None):
    d = p.d
    gb, sg, ob = p.gb, p.sg, p.ob

    nc = bacc.Bacc("TRN2", target_bir_lowering=False, debug=False,
                   num_devices=n_cores or p.n_cores)

    X = nc.dram_tensor("X", [p.n_pad, d], BF16, kind="ExternalInput")
    WT = nc.dram_tensor("WT", [d, d], BF16, kind="ExternalInput")
    iota = nc.dram_tensor("iota", [128, 128], BF16, kind="ExternalInput")
    dst_rel = nc.dram_tensor("dst_rel", [128, p.n_inst], BF16,
                             kind="ExternalInput")
    w_t = nc.dram_tensor("w_t", [128, p.n_inst], BF16, kind="ExternalInput")
    idx_t = [nc.dram_tensor(f"idx{t}", [128, max(p.stream_len[t] // 16, 1)],
                            I16, kind="ExternalInput")
             for t in range(p.n_streams)]
    out_t = nc.dram_tensor("out", [d, p.slots * 128], BF16,
                           kind="ExternalOutput")

    with tile.TileContext(nc) as tc, ExitStack() as ctx:
        cpool = ctx.enter_context(tc.tile_pool(name="consts", bufs=1))
        WT_sb = cpool.tile([d, d], BF16)
        nc.sync.dma_start(WT_sb[:], WT.ap()[:, :])
        iota_sb = cpool.tile([128, 128], BF16)
        nc.sync.dma_start(iota_sb[:], iota.ap()[:, :])
        dstrel_lo = cpool.tile([128, p.n_inst], BF16)
        nc.sync.dma_start(dstrel_lo[:], dst_rel.ap()[:, :])
        w_lo = cpool.tile([128, p.n_inst], BF16)
        nc.sync.dma_start(w_lo[:], w_t.ap()[:, :])
        dstrel_sb = cpool.tile([128, p.n_inst], F32)
        nc.vector.tensor_scalar_mul(dstrel_sb[:], dstrel_lo[:], 1.0)
        w_sb = cpool.tile([128, p.n_inst], F32)
        nc.vector.tensor_scalar_mul(w_sb[:], w_lo[:], 1.0)
        idx_sb = []
        for t in range(p.n_streams):
            it = cpool.tile([128, idx_t[t].shape[1]], I16, name=f"idxsb{t}")
            nc.sync.dma_start(it[:], idx_t[t].ap()[:, :])
            idx_sb.append(it)

        gpools = [ctx.enter_context(tc.tile_pool(name=f"g{q}", bufs=4))
                  for q in range(p.n_subt)]
        # per-stream pools for the self-pair gathers (all prefetched)
        spools = [ctx.enter_context(tc.tile_pool(name=f"gself{q}", bufs=2))
                  for q in range(p.n_subt)]
        stpool = ctx.enter_context(tc.tile_pool(name="st", bufs=24))
        aggpool = ctx.enter_context(tc.tile_pool(name="agg", bufs=3))
        opool = ctx.enter_context(tc.tile_pool(name="ostage", bufs=4))
        pa = ctx.enter_context(tc.tile_pool(name="pa", bufs=4, space="PSUM"))
        pb = ctx.enter_context(tc.tile_pool(name="pb", bufs=4, space="PSUM"))

        gtiles = {}
        colbatch = []
        for t in range(p.n_streams):
            cb = np.zeros(max(p.stream_len[t] // 128, 1), dtype=np.int64)
            for bi, (c0, ncols) in enumerate(p.batch_bounds[t]):
                cb[c0:c0 + ncols] = bi
            colbatch.append(cb)

        def get_gtile(t, i):
            if (t, i) not in gtiles:
                c0, ncols = p.batch_bounds[t][i]
                lanes = ncols * 128
                ew = p.stream_ew[t]
                q = p.stream_q[t]
                src = X.ap()[int(p.subt_off[q]):int(p.subt_off[q + 1]), :]
                if t >= p.n_subt:
                    src = src.rearrange("(r two) d -> r (two d)", two=2)
                    mx = max(nc2 for _, nc2 in p.batch_bounds[t])
                    gt = spools[q].tile([128, mx * ew], BF16, name="gts")
                else:
                    gt = gpools[q].tile([128, gb * ew], BF16, name=f"gt{q}")
                nc.gpsimd.dma_gather(
                    out_ap=gt[:, 0:ncols * ew].rearrange(
                        "p (j e) -> p j e", e=ew),
                    in_ap=src,
                    idxs_ap=idx_sb[t][:, c0 * 8: c0 * 8 + lanes // 16],
                    num_idxs=lanes,
                    num_idxs_reg=lanes,
                    elem_size=ew,
                    single_packet=False,
                )
                gtiles[(t, i)] = gt
            return gtiles[(t, i)]

        for t in range(p.n_subt, p.n_streams):
            for i in range(len(p.batch_bounds[t])):
                get_gtile(t, i)

        ostage = None
        for s in range(p.slots):
            ps_a = pa.tile([128, d], F32, name="pa_t", space="PSUM")
            i0, i1 = p.slot_inst_range[s], p.slot_inst_range[s + 1]
            for ii in range(i0, i1):
                # weighted one-hot: st[r, j] = (iota[j] == dst_rel[r]) * w[r]
                st_t = stpool.tile([128, 128], BF16, name="st_t")
                nc.vector.tensor_scalar(
                    out=st_t[:], in0=iota_sb[:],
                    scalar1=dstrel_sb[:, ii:ii + 1],
                    scalar2=w_sb[:, ii:ii + 1],
                    op0=mybir.AluOpType.is_equal,
                    op1=mybir.AluOpType.mult)
                t = int(p.inst_stream[ii])
                c = int(p.inst_col[ii])
                bi = int(colbatch[t][c])
                gt = get_gtile(t, bi)
                ew = p.stream_ew[t]
                o = (c - p.batch_bounds[t][bi][0]) * ew \
                    + int(p.inst_half[ii]) * d
                nc.tensor.matmul(out=ps_a[:], lhsT=gt[:, o:o + d],
                                 rhs=st_t[:],
                                 start=(ii == i0), stop=(ii == i1 - 1))
            agg = aggpool.tile([128, d], BF16, name="agg_t")
            if s % 2 == 0:
                nc.scalar.activation(agg[:], ps_a[:],
                                     mybir.ActivationFunctionType.Copy)
            else:
                nc.vector.tensor_scalar_mul(agg[:], ps_a[:], 1.0)
            ps_b = pb.tile([128, d], F32, name="pb_t", space="PSUM")
            nc.tensor.matmul(out=ps_b[:], lhsT=WT_sb[:], rhs=agg[:],
                             start=True, stop=True)
            if s % ob == 0:
                ostage = opool.tile([128, ob * d], BF16, name="ostage")
            ocol = (s % ob) * d
            if s % 2 == 0:
                nc.vector.tensor_scalar_mul(ostage[:, ocol:ocol + d],
                                            ps_b[:], 1.0)
            else:
                nc.scalar.activation(ostage[:, ocol:ocol + d], ps_b[:],
                                     mybir.ActivationFunctionType.Copy)
            if s % ob == ob - 1 or s == p.slots - 1:
                s0 = s // ob * ob
                nsw = s - s0 + 1
                nc.sync.dma_start(
                    out=out_t.ap()[:, s0 * 128:(s0 + nsw) * 128],
                    in_=ostage[:, 0:nsw * d])

    nc.compile()
    return nc


# ---------------------------------------------------------------------------
# Orchestration
# ---------------------------------------------------------------------------


def make_inputs(p, x, W):
    d = p.d
    Xb = np.zeros((p.n_pad, d), dtype=ml_dtypes.bfloat16)
    Xb[:p.n_nodes] = np.asarray(x, dtype=np.float32).astype(ml_dtypes.bfloat16)
    WT = np.ascontiguousarray(np.asarray(W, dtype=np.float32).T).astype(
        ml_dtypes.bfloat16)
    iota = np.broadcast_to(np.arange(128, dtype=np.float32),
                           (128, 128)).astype(ml_dtypes.bfloat16).copy()
    common = {"X": Xb, "WT": WT, "iota": iota}
    in_maps = []
    for c in range(p.n_cores):
        m = dict(common)
        m["dst_rel"] = p.core_dst_rel[c]
        m["w_t"] = p.core_w[c]
        for t in range(p.n_streams):
            m[f"idx{t}"] = p.core_idx[c][t]
        in_maps.append(m)
    return in_maps


def assemble_output(p, results, b):
    out = np.zeros((p.n_nodes, p.d), dtype=np.float32)
    for c in range(p.n_cores):
        oc = np.asarray(results[c]["out"], dtype=np.float32)  # [d, slots*128]
        for s, blk in enumerate(p.core_blocks[c]):
            lo = blk * 128
            if lo >= p.n_nodes:
                continue
            hi = min(lo + 128, p.n_nodes)
            out[lo:hi] = oc[:, s * 128: s * 128 + (hi - lo)].T
    # bias epilogue: out[v] += rdeg[v] * sraw[v] * b
    coef = (p.rdeg[:p.n_nodes] * p.sraw[:p.n_nodes]).astype(np.float32)
    out += coef[:, None] * np.asarray(b, dtype=np.float32)[None, :]
    return out


def gcn_forward(x, edge_index, W, b, n_cores=8, trace=False, **plan_kw):
    n = x.shape[0]
    src = np.asarray(edge_index[0])
    dst = np.asarray(edge_index[1])
    p = build_plan(src, dst, n, n_cores, d=W.shape[0], **plan_kw)
    nc = build_nc(p)
    in_maps = make_inputs(p, x, W)
    res = run_bass_kernel_spmd(nc, in_maps, core_ids=list(range(n_cores)),
                               trace=trace)
    out = assemble_output(p, [r for r in res.results], b)
    return out, p, res


def kernel(x, edge_index, W, b):
    """GCN layer forward on 8 trn2 NeuronCores. Inputs as in setup_inputs()."""
    x = np.asarray(x, dtype=np.float32)
    edge_index = np.asarray(edge_index)
    W = np.asarray(W, dtype=np.float32)
    b = np.asarray(b, dtype=np.float32)
    out, _p, _res = gcn_forward(x, edge_index, W, b, n_cores=N_CORES)
    return out.astype(np.float32)


# revision 38
# speedup vs baseline: 1.9973x; 1.0075x over previous
"""GCN layer (out = D^-1/2 (A+I) D^-1/2 (x W^T + b)) on 8 trn2 NeuronCores.

Strategy (direct-x aggregation, no intermediate table):
  out[dst] = W @ (sum_e norm_e * x[src_e]) + (sum_e norm_e) * b
  with norm_e = rdeg[src]*rdeg[dst].

  - Host: append self-loops, sort edges by dst, partition dst-blocks (128
    nodes) across 8 cores (LPT), build a core-invariant static schedule:
    per (slot, subtable) segments padded at LANE level to the max across
    cores (uniform SPMD program; per-core idx/weight tables).
  - Device: gather raw x rows (bf16, 256B) per edge lane straight from the
    uploaded input table (4 int16-indexable subtable views); accumulate
    aggT[din, dst] = sum_lane x_lane[din] * S[lane, dst] on the PE, where
    S = weighted one-hot (weight = norm_e, built by DVE is_equal * w);
    then one W matmul per dst block; bias term applied on host
    (out += rdeg*sraw*b, with sraw = sum_e rdeg[src]).

  Self-loops are extracted from the edge streams into pair-gathers: each
  block's 128 own nodes are consecutive X rows, fetched as 64 two-node
  512B descriptors (no <512B DMA penalty -> half cost per node) and folded
  in via two extra matmuls per slot.
"""

import math
import time
from contextlib import ExitStack

import ml_dtypes
import numpy as np

import concourse.bass as bass
import concourse.tile as tile
from concourse import bacc, mybir
from concourse.bass_utils import run_bass_kernel_spmd

F32 = mybir.dt.float32
BF16 = mybir.dt.bfloat16
I16 = mybir.dt.int16

N_NODES = 100000
N_EDGES = 1600000
IN_CH = 128
OUT_CH = 128
N_CORES = 8

# ---------------------------------------------------------------------------
# Host-side planning
# ---------------------------------------------------------------------------


class Plan:
    pass


def _optimize_groups(blk_q_cnt, groups, home, n_cores, rounds=60,
                     batch=16384, seed=7):
    """Vectorized local search: swap blocks between slot-groups of the same
    home class to reduce sum over groups of sum_q max (padded lane count)."""
    rng = np.random.default_rng(seed)
    gmat = np.stack([np.asarray(g) for g in groups])          # [G, C]
    G, C = gmat.shape
    ghome = home[gmat[:, 0]]
    obj = blk_q_cnt[gmat].max(axis=1).sum(axis=1)             # [G]
    byhome = {}
    for i in range(G):
        byhome.setdefault(int(ghome[i]), []).append(i)
    classes = [np.asarray(v) for v in byhome.values() if len(v) > 1]
    if not classes:
        return [gmat[i] for i in range(G)]
    for _ in range(rounds):
        # class-matched random group pairs
        cls = [c[rng.integers(0, len(c), size=(batch, 2))] for c in classes]
        pij = np.concatenate(cls, axis=0)
        keep = pij[:, 0] != pij[:, 1]
        pij = pij[keep]
        K = len(pij)
        ab = rng.integers(0, C, size=(K, 2))
        gi = gmat[pij[:, 0]].copy()                           # [K, C]
        gj = gmat[pij[:, 1]].copy()
        bi = gj[np.arange(K), ab[:, 1]].copy()
        gj[np.arange(K), ab[:, 1]] = gi[np.arange(K), ab[:, 0]]
        gi[np.arange(K), ab[:, 0]] = bi
        oi = blk_q_cnt[gi].max(axis=1).sum(axis=1)
        oj = blk_q_cnt[gj].max(axis=1).sum(axis=1)
        delta = (oi + oj) - (obj[pij[:, 0]] + obj[pij[:, 1]])
        good = np.where(delta < 0)[0]
        if len(good) == 0:
            continue
        good = good[np.argsort(delta[good])]
        used = np.zeros(G, dtype=bool)
        for k in good:
            i, j = pij[k]
            if used[i] or used[j]:
                continue
            used[i] = used[j] = True
            gmat[i] = gi[k]
            gmat[j] = gj[k]
            obj[i] = oi[k]
            obj[j] = oj[k]
    return [gmat[i] for i in range(G)]


def build_plan(src, dst, n_nodes, n_cores, d=128, gb=14, sg=8, ob=16,
               subt_cap=32768):
    """src/dst: REAL edge endpoints (self loops handled separately)."""
    t0 = time.time()
    p = Plan()
    p.d = d
    p.gb = gb
    p.sg = sg
    p.ob = ob
    p.n_nodes = n_nodes
    p.n_cores = n_cores

    n_blocks = math.ceil(n_nodes / 128)
    slots = math.ceil(n_blocks / n_cores)
    n_blocks = slots * n_cores
    p.slots = slots
    p.n_blocks = n_blocks
    p.n_pad = n_blocks * 128

    # subtables over node id range (int16 gather index limit)
    n_subt = math.ceil(p.n_pad / subt_cap)
    off = [min(q * subt_cap, p.n_pad) for q in range(n_subt + 1)]
    p.subt_off = np.asarray(off)
    p.n_subt = n_subt

    src = np.asarray(src, dtype=np.int64)
    dst = np.asarray(dst, dtype=np.int64)
    loop = np.arange(n_nodes, dtype=np.int64)

    # degrees / norms (host; exactly mirrors the reference formula,
    # self-loops included in the degree)
    deg = np.bincount(np.concatenate([dst, loop]),
                      minlength=p.n_pad).astype(np.float64)
    deg[n_nodes:] = 1.0
    deg[deg == 0] = 1.0
    rdeg = 1.0 / np.sqrt(deg)
    p.rdeg = rdeg.astype(np.float32)
    norm_real = (rdeg[src] * rdeg[dst]).astype(np.float32)
    # host bias epilogue: out[v] += rdeg[v] * sraw[v] * b  (incl. self loop)
    sraw = np.zeros(p.n_pad, dtype=np.float64)
    np.add.at(sraw, dst, rdeg[src])
    sraw[:n_nodes] += rdeg[:n_nodes]
    p.sraw = sraw.astype(np.float32)
    # self-loop weight per node (x[v] contribution), 0 for pad nodes
    wself = (rdeg * rdeg).astype(np.float32)
    wself[n_nodes:] = 0.0
    p.wself = wself

    # sort REAL edges by dst
    order = np.argsort(dst, kind="stable")
    dst_s = dst[order]
    src_s = src[order]
    norm_s = norm_real[order]
    blk_ptr = np.searchsorted(dst_s, np.arange(0, p.n_pad + 1, 128))
    blk_cnt = blk_ptr[1:] - blk_ptr[:-1]

    # per (block, q) edge sublists
    subt_of_src = np.searchsorted(p.subt_off[1:], src_s, side="right")
    blk_edges = []  # [block][q] -> (src_local, dst_local, norm)
    blk_q_cnt = np.zeros((n_blocks, n_subt), dtype=np.int64)
    for b in range(n_blocks):
        lo, hi = blk_ptr[b], blk_ptr[b + 1]
        qs = subt_of_src[lo:hi]
        per_q = []
        for q in range(n_subt):
            m = qs == q
            sl = (src_s[lo:hi][m] - p.subt_off[q]).astype(np.int16)
            dl = (dst_s[lo:hi][m] - b * 128).astype(np.int32)
            nm = norm_s[lo:hi][m]
            per_q.append((sl, dl, nm))
            blk_q_cnt[b, q] = len(sl)
        blk_edges.append(per_q)

    # Slot grouping: blocks sharing a slot should have near-identical per-q
    # edge counts, since seg_len[s][q] = max over the group. Self-loops
    # concentrate ~128 edges in a block's home subtable, so group by
    # (home subtable, size rank); within a slot, biggest block goes to the
    # least-loaded core (balances per-core totals).
    home = np.searchsorted(p.subt_off[1:], np.arange(n_blocks) * 128,
                           side="right")
    groups = []
    for q in range(n_subt):
        hb = np.where(home == q)[0]
        hb = hb[np.argsort(-blk_cnt[hb], kind="stable")]
        assert len(hb) % n_cores == 0, (q, len(hb))
        for i in range(0, len(hb), n_cores):
            groups.append(hb[i:i + n_cores])
    assert len(groups) == slots
    groups = _optimize_groups(blk_q_cnt, groups, home, n_cores)
    core_loads = np.zeros(n_cores, dtype=np.int64)
    core_blocks = [[-1] * slots for _ in range(n_cores)]
    for s, group in enumerate(groups):
        free = list(range(n_cores))
        group = sorted(group, key=lambda b: -int(blk_cnt[b]))
        for b in group:  # desc by size
            c = min(free, key=lambda c: core_loads[c])
            free.remove(c)
            core_loads[c] += int(blk_cnt[b])
            core_blocks[c][s] = int(b)
    p.core_blocks = core_blocks

    # Streams 0..n_subt-1: per-edge gathers (elem = d, one idx per edge).
    # Streams n_subt..2*n_subt-1: self-loop pair gathers (elem = 2d, one idx
    # per CONSECUTIVE NODE PAIR of the slot's own block; 64 lanes per slot,
    # in the block's home subtable). 512B descriptors avoid the <512B DMA
    # penalty, halving the per-node self-loop gather cost.
    n_streams = 2 * n_subt
    p.n_streams = n_streams
    group_home = np.zeros(slots, dtype=np.int64)
    for s in range(slots):
        group_home[s] = home[core_blocks[0][s]]
    p.group_home = group_home
    seg_len = np.zeros((slots, n_streams), dtype=np.int64)
    for c in range(n_cores):
        for s in range(slots):
            seg_len[s, :n_subt] = np.maximum(seg_len[s, :n_subt],
                                             blk_q_cnt[core_blocks[c][s]])
    for s in range(slots):
        seg_len[s, n_subt + group_home[s]] = 64
    p.seg_len = seg_len
    seg_start = np.zeros((slots, n_streams), dtype=np.int64)
    cur = np.zeros(n_streams, dtype=np.int64)
    for s in range(slots):
        seg_start[s] = cur
        cur += seg_len[s]
    p.seg_start = seg_start
    stream_len = [int(-(-int(cur[t]) // 128) * 128) for t in range(n_streams)]
    p.stream_len = stream_len
    p.stream_q = [t % n_subt for t in range(n_streams)]
    p.stream_ew = [d if t < n_subt else 2 * d for t in range(n_streams)]

    # gather batch boundaries per stream, in columns. Full gb-col batches,
    # but ramp the tail down so the post-last-gather matmul drain is short.
    p.batch_bounds = []  # [stream] -> list of (col0, ncols)
    for t in range(n_streams):
        cols = stream_len[t] // 128
        bounds = []
        c0 = 0
        if t >= n_subt:
            # self-pair streams: halve into <=2 prefetchable batches
            if cols > 0:
                h = -(-cols // 2)
                bounds.append((0, h))
                if cols - h > 0:
                    bounds.append((h, cols - h))
            p.batch_bounds.append(bounds)
            continue
        while cols - c0 > gb:
            bounds.append((c0, gb))
            c0 += gb
        r = cols - c0
        while r > 6:
            tt = max(6, -(-r // 2))
            bounds.append((c0, tt))
            c0 += tt
            r -= tt
        if r > 0:
            bounds.append((c0, r))
        p.batch_bounds.append(bounds)

    # instance enumeration (program order: slot-major; self pairs first,
    # then per-edge gather columns)
    insts = []  # (slot, stream, col, half)
    slot_inst_range = [0]
    inst_index = {}
    for s in range(slots):
        cnt = 0
        sp = n_subt + int(group_home[s])
        L0 = int(seg_start[s][sp])
        assert seg_len[s][sp] == 64 and L0 % 64 == 0
        col = L0 // 128
        for half in (0, 1):
            inst_index[(s, sp, col, half)] = len(insts)
            insts.append((s, sp, col, half))
            cnt += 2
        for q in range(n_subt):
            L0 = int(seg_start[s][q])
            L1 = L0 + int(seg_len[s][q])
            if L1 > L0:
                for col in range(L0 // 128, -(-L1 // 128)):
                    inst_index[(s, q, col, 0)] = len(insts)
                    insts.append((s, q, col, 0))
                    cnt += 1
        slot_inst_range.append(len(insts))
    p.n_inst = len(insts)
    p.inst_slot = np.asarray([i[0] for i in insts], dtype=np.int64)
    p.inst_stream = np.asarray([i[1] for i in insts], dtype=np.int64)
    p.inst_col = np.asarray([i[2] for i in insts], dtype=np.int64)
    p.inst_half = np.asarray([i[3] for i in insts], dtype=np.int64)
    p.slot_inst_range = slot_inst_range

    # per-core tables
    p.core_idx = []      # [n_cores][stream] int16 wrapped [128, len/16]
    p.core_dst_rel = []  # [n_cores] bf16 [128, n_inst]
    p.core_w = []        # [n_cores] bf16 [128, n_inst]
    for c in range(n_cores):
        idx_t = [np.zeros(stream_len[t], dtype=np.int16)
                 for t in range(n_streams)]
        dst_rel = np.full((128, p.n_inst), -1.0, dtype=np.float32)
        w_lane = np.zeros((128, p.n_inst), dtype=np.float32)
        for s in range(slots):
            b = core_blocks[c][s]
            for q in range(n_subt):
                sl, dl, nm = blk_edges[b][q]
                n_e = len(sl)
                if n_e == 0:
                    continue
                L0 = int(seg_start[s][q])
                idx_t[q][L0:L0 + n_e] = sl
                lanes = L0 + np.arange(n_e)
                cols = lanes // 128
                rows = lanes % 128
                iis = np.asarray(
                    [inst_index[(s, q, int(cc), 0)] for cc in cols],
                    dtype=np.int64)
                dst_rel[rows, iis] = dl
                w_lane[rows, iis] = nm
            # self-loop pairs: 64 lanes, block rows are consecutive in X
            sp = n_subt + int(group_home[s])
            hq = int(group_home[s])
            assert home[b] == hq
            L0 = int(seg_start[s][sp])
            base_pair = (b * 128 - int(p.subt_off[hq])) // 2
            idx_t[sp][L0:L0 + 64] = base_pair + np.arange(64)
            rows = (L0 + np.arange(64)) % 128
            for half in (0, 1):
                ii = inst_index[(s, sp, L0 // 128, half)]
                dst_rel[rows, ii] = 2 * np.arange(64) + half
                w_lane[rows, ii] = p.wself[b * 128 + 2 * np.arange(64) + half]
        idx_wrapped = []
        for t in range(n_streams):
            if stream_len[t] == 0:
                idx_wrapped.append(np.zeros((128, 1), dtype=np.int16))
                continue
            a = idx_t[t].reshape(-1, 16).T  # [16, L/16]
            idx_wrapped.append(np.tile(a, (8, 1)).copy())
        p.core_idx.append(idx_wrapped)
        p.core_dst_rel.append(dst_rel.astype(ml_dtypes.bfloat16))
        p.core_w.append(w_lane.astype(ml_dtypes.bfloat16))

    p.plan_time = time.time() - t0
    return p


# ---------------------------------------------------------------------------
# Device kernel
# ---------------------------------------------------------------------------


def build_nc(p, n_cores=None):
    d = p.d
    gb, sg, ob = p.gb, p.sg, p.ob

    nc = bacc.Bacc("TRN2", target_bir_lowering=False, debug=False,
                   num_devices=n_cores or p.n_cores)

    X = nc.dram_tensor("X", [p.n_pad, d], BF16, kind="ExternalInput")
    WT = nc.dram_tensor("WT", [d, d], BF16, kind="ExternalInput")
    iota = nc.dram_tensor("iota", [128, 128], BF16, kind="ExternalInput")
    dst_rel = nc.dram_tensor("dst_rel", [128, p.n_inst], BF16,
                             kind="ExternalInput")
    w_t = nc.dram_tensor("w_t", [128, p.n_inst], BF16, kind="ExternalInput")
    idx_t = [nc.dram_tensor(f"idx{t}", [128, max(p.stream_len[t] // 16, 1)],
                            I16, kind="ExternalInput")
             for t in range(p.n_streams)]
    out_t = nc.dram_tensor("out", [d, p.slots * 128], BF16,
                           kind="ExternalOutput")

    with tile.TileContext(nc) as tc, ExitStack() as ctx:
        cpool = ctx.enter_context(tc.tile_pool(name="consts", bufs=1))
        idx_sb = []
        for t in range(p.n_streams):
            it = cpool.tile([128, idx_t[t].shape[1]], I16, name=f"idxsb{t}")
            nc.sync.dma_start(it[:], idx_t[t].ap()[:, :])
            idx_sb.append(it)
        dstrel_lo = cpool.tile([128, p.n_inst], BF16)
        nc.sync.dma_start(dstrel_lo[:], dst_rel.ap()[:, :])
        w_lo = cpool.tile([128, p.n_inst], BF16)
        nc.sync.dma_start(w_lo[:], w_t.ap()[:, :])
        WT_sb = cpool.tile([d, d], BF16)
        nc.sync.dma_start(WT_sb[:], WT.ap()[:, :])
        iota_sb = cpool.tile([128, 128], BF16)
        nc.sync.dma_start(iota_sb[:], iota.ap()[:, :])
        dstrel_sb = cpool.tile([128, p.n_inst], F32)
        nc.vector.tensor_scalar_mul(dstrel_sb[:], dstrel_lo[:], 1.0)
        w_sb = cpool.tile([128, p.n_inst], F32)
        nc.vector.tensor_scalar_mul(w_sb[:], w_lo[:], 1.0)

        gpools = [ctx.enter_context(tc.tile_pool(name=f"g{q}", bufs=4))
                  for q in range(p.n_subt)]
        # per-stream pools for the self-pair gathers (all prefetched)
        spools = [ctx.enter_context(tc.tile_pool(name=f"gself{q}", bufs=2))
                  for q in range(p.n_subt)]
        stpool = ctx.enter_context(tc.tile_pool(name="st", bufs=24))
        aggpool = ctx.enter_context(tc.tile_pool(name="agg", bufs=6))
        opool = ctx.enter_context(tc.tile_pool(name="ostage", bufs=4))
        pa = ctx.enter_context(tc.tile_pool(name="pa", bufs=4, space="PSUM"))
        pb = ctx.enter_context(tc.tile_pool(name="pb", bufs=4, space="PSUM"))

        gtiles = {}
        colbatch = []
        for t in range(p.n_streams):
            cb = np.zeros(max(p.stream_len[t] // 128, 1), dtype=np.int64)
            for bi, (c0, ncols) in enumerate(p.batch_bounds[t]):
                cb[c0:c0 + ncols] = bi
            colbatch.append(cb)

        def get_gtile(t, i):
            if (t, i) not in gtiles:
                c0, ncols = p.batch_bounds[t][i]
                lanes = ncols * 128
                ew = p.stream_ew[t]
                q = p.stream_q[t]
                src = X.ap()[int(p.subt_off[q]):int(p.subt_off[q + 1]), :]
                if t >= p.n_subt:
                    src = src.rearrange("(r two) d -> r (two d)", two=2)
                    mx = max(nc2 for _, nc2 in p.batch_bounds[t])
                    gt = spools[q].tile([128, mx * ew], BF16, name="gts")
                else:
                    gt = gpools[q].tile([128, gb * ew], BF16, name=f"gt{q}")
                nc.gpsimd.dma_gather(
                    out_ap=gt[:, 0:ncols * ew].rearrange(
                        "p (j e) -> p j e", e=ew),
                    in_ap=src,
                    idxs_ap=idx_sb[t][:, c0 * 8: c0 * 8 + lanes // 16],
                    num_idxs=lanes,
                    num_idxs_reg=lanes,
                    elem_size=ew,
                    single_packet=False,
                )
                gtiles[(t, i)] = gt
            return gtiles[(t, i)]

        for t in range(p.n_subt, p.n_streams):
            for i in range(len(p.batch_bounds[t])):
                get_gtile(t, i)

        ostage = None
        for s in range(p.slots):
            ps_a = pa.tile([128, d], F32, name="pa_t", space="PSUM")
            i0, i1 = p.slot_inst_range[s], p.slot_inst_range[s + 1]
            for ii in range(i0, i1):
                # weighted one-hot: st[r, j] = (iota[j] == dst_rel[r]) * w[r]
                st_t = stpool.tile([128, 128], BF16, name="st_t")
                nc.vector.tensor_scalar(
                    out=st_t[:], in0=iota_sb[:],
                    scalar1=dstrel_sb[:, ii:ii + 1],
                    scalar2=w_sb[:, ii:ii + 1],
                    op0=mybir.AluOpType.is_equal,
                    op1=mybir.AluOpType.mult)
                t = int(p.inst_stream[ii])
                c = int(p.inst_col[ii])
                bi = int(colbatch[t][c])
                gt = get_gtile(t, bi)
                ew = p.stream_ew[t]
                o = (c - p.batch_bounds[t][bi][0]) * ew \
                    + int(p.inst_half[ii]) * d
                nc.tensor.matmul(out=ps_a[:], lhsT=gt[:, o:o + d],
                                 rhs=st_t[:],
                                 start=(ii == i0), stop=(ii == i1 - 1))
            agg = aggpool.tile([128, d], BF16, name="agg_t")
            if s % 2 == 0:
                nc.scalar.activation(agg[:], ps_a[:],
                                     mybir.ActivationFunctionType.Copy)
            else:
                nc.vector.tensor_scalar_mul(agg[:], ps_a[:], 1.0)
            ps_b = pb.tile([128, d], F32, name="pb_t", space="PSUM")
            nc.tensor.matmul(out=ps_b[:], lhsT=WT_sb[:], rhs=agg[:],
                             start=True, stop=True)
            if s % ob == 0:
                ostage = opool.tile([128, ob * d], BF16, name="ostage")
            ocol = (s % ob) * d
            if s % 2 == 0:
                nc.vector.tensor_scalar_mul(ostage[:, ocol:ocol + d],
                                            ps_b[:], 1.0)
            else:
                nc.scalar.activation(ostage[:, ocol:ocol + d], ps_b[:],
                                     mybir.ActivationFunctionType.Copy)
            if s % ob == ob - 1 or s == p.slots - 1:
                s0 = s // ob * ob
                nsw = s - s0 + 1
                nc.sync.dma_start(
                    out=out_t.ap()[:, s0 * 128:(s0 + nsw) * 128],
                    in_=ostage[:, 0:nsw * d])

    nc.compile()
    return nc


# ---------------------------------------------------------------------------
# Orchestration
# ---------------------------------------------------------------------------


def make_inputs(p, x, W):
    d = p.d
    Xb = np.zeros((p.n_pad, d), dtype=ml_dtypes.bfloat16)
    Xb[:p.n_nodes] = np.asarray(x, dtype=np.float32).astype(ml_dtypes.bfloat16)
    WT = np.ascontiguousarray(np.asarray(W, dtype=np.float32).T).astype(
        ml_dtypes.bfloat16)
    iota = np.broadcast_to(np.arange(128, dtype=np.float32),
                           (128, 128)).astype(ml_dtypes.bfloat16).copy()
    common = {"X": Xb, "WT": WT, "iota": iota}
    in_maps = []
    for c in range(p.n_cores):
        m = dict(common)
        m["dst_rel"] = p.core_dst_rel[c]
        m["w_t"] = p.core_w[c]
        for t in range(p.n_streams):
            m[f"idx{t}"] = p.core_idx[c][t]
        in_maps.append(m)
    return in_maps


def assemble_output(p, results, b):
    out = np.zeros((p.n_nodes, p.d), dtype=np.float32)
    for c in range(p.n_cores):
        oc = np.asarray(results[c]["out"], dtype=np.float32)  # [d, slots*128]
        for s, blk in enumerate(p.core_blocks[c]):
            lo = blk * 128
            if lo >= p.n_nodes:
                continue
            hi = min(lo + 128, p.n_nodes)
            out[lo:hi] = oc[:, s * 128: s * 128 + (hi - lo)].T
    # bias epilogue: out[v] += rdeg[v] * sraw[v] * b
    coef = (p.rdeg[:p.n_nodes] * p.sraw[:p.n_nodes]).astype(np.float32)
    out += coef[:, None] * np.asarray(b, dtype=np.float32)[None, :]
    return out


def gcn_forward(x, edge_index, W, b, n_cores=8, trace=False, **plan_kw):
    n = x.shape[0]
    src = np.asarray(edge_index[0])
    dst = np.asarray(edge_index[1])
    p = build_plan(src, dst, n, n_cores, d=W.shape[0], **plan_kw)
    nc = build_nc(p)
    in_maps = make_inputs(p, x, W)
    res = run_bass_kernel_spmd(nc, in_maps, core_ids=list(range(n_cores)),
                               trace=trace)
    out = assemble_output(p, [r for r in res.results], b)
    return out, p, res


def kernel(x, edge_index, W, b):
    """GCN layer forward on 8 trn2 NeuronCores. Inputs as in setup_inputs()."""
    x = np.asarray(x, dtype=np.float32)
    edge_index = np.asarray(edge_index)
    W = np.asarray(W, dtype=np.float32)
    b = np.asarray(b, dtype=np.float32)
    out, _p, _res = gcn_forward(x, edge_index, W, b, n_cores=N_CORES)
    return out.astype(np.float32)
